# revision 1
# baseline (speedup 1.0000x reference)
"""Trainium2 Bass kernel for nn_FCGAT (fully-connected GAT variant).

Mathematical simplifications used (exact, not approximate):

1. The reference computes
   ``out = einsum('nkj,nkd->nkd', softmax(aa, axis=2), h)`` which is
   ``h[n,k,d] * sum_j softmax(aa)[n,k,j] == h[n,k,d]``.  The whole attention
   block (z tensor, aw1/ab1/aw2/ab2, softmax) is dead code in real
   arithmetic; only float rounding noise (~1e-14 rel) distinguishes it.
   The model reduces to, per step::

       h_s = lrelu(lrelu([towers | x_s] @ w1.T + b1) @ w2.T + b2)
       x_{s+1} = h_s + x_s

   followed by ``prod_k sigmoid(x_K @ ow[0] + ob[0])`` over the K nodes of
   each graph.

2. The residual is distributed through the (linear) first matmul of the
   next step: ``x_s = x_0 + sum_{t<s} h_t``, so

       mm1(s) = w1 @ [towers | x_0]  +  sum_{t<s} w1[:, DT:] @ h_t
       logits = ow @ x_0 + sum_t ow @ h_t

   both as PSUM accumulation groups.  x_s is never materialized and there
   are no elementwise adds at all.

Sharding: data-parallel over the batch dim N=128 -> 16 graphs (1024 rows)
per core across 8 NeuronCores; all weights replicated.

Written in raw Bass (explicit engine blocks + semaphores) rather than Tile:
this toolchain's walrus build allows only ONE sync-wait command per
instruction, and Tile's auto-generated synchronization (per-DMA-queue
semaphores, slot-release waits, the kernel-tail drain) routinely needs
several.  Raw Bass sidesteps all of it: standalone single-condition wait
instructions and happens-before transitivity applied by construction.
Each DMA gets its own semaphore waited at full value only (concurrent DMA
completions are unordered, so intermediate shared-sem values are racy).

Performance notes:
  * linear-layer matmuls run as float32r (1 PE cycle/row at >=256 moving
    rows vs fp32's 4); every producer of a matmul operand writes a
    float32r-rounded output, which the BIR verifier requires;
  * the leaky_relu and sigmoid ACT tables never share an act-func-set
    (~1.3us load each), so both are prewarmed by dummy ops during the DMA
    phase;
  * the x0-part matmul of each accumulation group issues before the h2
    wait (only the h2-term matmul waits), hiding it under the eviction;
  * the two logits groups use separate PSUM banks (chunk 1 reuses the
    weight-transpose bank, long dead by then) so they don't serialize
    against the first sigmoid;
  * per-graph product = log2(K) tree of strided DVE multiplies (DVE
    TensorReduce has no mult op).

On-chip layout is feature-major ([feature, row] on partitions) so the
linear layers contract over the partition dim on the tensor engine.  Rows
are transposed on device with PE transposes (fp32 has no DMA transpose).
w1/w2 are pre-transposed on host inside the packed-constants array; the
DVE gives them (and ow) the float32r-rounding pass the verifier requires.
All constants arrive in one packed DMA on the SP HWDGE queue; x/towers
rows arrive in 4 two-tile DMAs split across the SP and ACT HWDGE queues.
"""

from contextlib import ExitStack

import numpy as np

import concourse.bass as bass
import concourse.mybir as mybir
from concourse.bass_utils import run_bass_kernel_spmd

N_CORES = 8
N, K, DT, D2 = 128, 64, 64, 64
D1 = DT + D2                # 128: [towers | x] feature dim
G = N // N_CORES            # 16 graphs per core
R = G * K                   # 1024 rows per core
CHUNK = 512                 # fp32 matmul moving-operand max
NCHUNK = R // CHUNK
PTILE = 128                 # rows per transpose tile
NTILE = R // PTILE
TILES_PER_DMA = 2
NDATA_DMA = NTILE // TILES_PER_DMA

# packed constants layout (columns of a [128, CW] f32 array)
C_ID = 0                    # 0:128    identity
C_W1 = 128                  # 128:256  w1.T (pre-transposed [d, o])
C_W2 = 256                  # 256:384  [0 | w2.T] (cols DT:D1 = w2.T)
C_B1 = 384                  # b1 column
C_B2 = 385                  # b2 in rows 64:128
C_OW = 386                  # ow[0] in rows 64:128, ob[0] at row 0
CW = 387

_F32 = mybir.dt.float32

# Results of the last hardware run (for the local test harness; the grading
# path only uses the return value of kernel()).
LAST_RESULT = None

_PROGRAM_CACHE = {}


def _build_program(kk: int, act_fn=None, use_fp32r=True) -> bass.Bass:
    LRELU = act_fn or mybir.ActivationFunctionType.Lrelu
    SIGMOID = mybir.ActivationFunctionType.Sigmoid

    def _r(ap):
        # float32r: same 4-byte storage; PE runs 1 cycle/row at >=256
        # moving rows instead of fp32's 4 (reduced internal precision).
        return ap.bitcast(mybir.dt.float32r) if use_fp32r else ap

    nc = bass.Bass()
    const_d = nc.declare_dram_parameter("cpack", [128, CW], _F32, isOutput=False)
    xc0_d = nc.declare_dram_parameter("xc0", [R, D1], _F32, isOutput=False)
    out_d = nc.declare_dram_parameter("out", [1, G], _F32, isOutput=True)

    # ---- instruction numbering (semaphore values), computed up front ----
    # PE: T_0..T_{NTILE-1}, then per step s: NCHUNK mm1 groups of (1+s)
    # matmuls followed by NCHUNK mm2s, then NCHUNK logits groups.
    pe = NTILE
    pe_mm1 = {}
    pe_mm2 = {}
    pe_mm3 = {}
    for s in range(kk):
        for c in range(NCHUNK):
            pe += 1 + s
            pe_mm1[(s, c)] = pe
        for c in range(NCHUNK):
            pe += 1
            pe_mm2[(s, c)] = pe
    for c in range(NCHUNK):
        pe += 1 + kk
        pe_mm3[c] = pe

    # ACT: two table-prewarm dummies (sigmoid, lrelu), then per step s:
    # h1(s,0..), h2(s,0..); finally sig(0..)
    ACT0 = 2

    def act_h1(s, c):
        return ACT0 + 2 * s * NCHUNK + c + 1

    def act_h2(s, c):
        return ACT0 + (2 * s + 1) * NCHUNK + c + 1

    def act_sig(c):
        return ACT0 + 2 * kk * NCHUNK + c + 1

    # DVE: w1t(1), w2t(2), ow-round(3), xcT 512-wide copies, then a
    # log2(K) tree of strided multiplies for the per-graph product
    NXCOPY = R // 512
    dve_xct = {c: 4 + (c * CHUNK) // 512 for c in range(NCHUNK)}
    NPROD = K.bit_length() - 1
    dve_prod = 3 + NXCOPY + 2 * NPROD

    with ExitStack() as ctx:
        cs = ctx.enter_context(nc.sbuf_tensor([128, CW], _F32))
        w1t = ctx.enter_context(nc.sbuf_tensor([D1, D1], _F32))
        w2t = ctx.enter_context(nc.sbuf_tensor([D1, D1], _F32))
        owf = ctx.enter_context(nc.sbuf_tensor([D1, 1], _F32))
        ldx = ctx.enter_context(nc.sbuf_tensor([PTILE, R], _F32))
        xcT = ctx.enter_context(nc.sbuf_tensor([D1, R], _F32))
        h1s = ctx.enter_context(
            nc.sbuf_tensor([D1, kk * NCHUNK * CHUNK + 1], _F32))
        h2s = ctx.enter_context(
            nc.sbuf_tensor([D1, kk * NCHUNK * CHUNK + 1], _F32))
        sig = ctx.enter_context(nc.sbuf_tensor([1, R], _F32))
        ptree = ctx.enter_context(nc.sbuf_tensor([1, R], _F32))
        prod = ctx.enter_context(nc.sbuf_tensor([1, G], _F32))
        warm = ctx.enter_context(nc.sbuf_tensor([1, 2], _F32))
        pst = ctx.enter_context(nc.psum_tensor([D1, R], _F32))
        ps_w = ctx.enter_context(nc.psum_tensor([128, 512], _F32))
        # full-bank allocations (a half-bank tensor could share a bank
        # with its neighbour -> fatal same-bank PE-write/engine-read overlap);
        # chunks rotate over two banks by parity
        ps1 = [ctx.enter_context(nc.psum_tensor(f"ps1_{p}", [D1, 512], _F32))
               for p in range(2)]
        ps2 = [ctx.enter_context(nc.psum_tensor(f"ps2_{p}", [D1, 512], _F32))
               for p in range(2)]
        ps3_0 = ctx.enter_context(nc.psum_tensor([1, 512], _F32))
        sem_const = ctx.enter_context(nc.semaphore("sem_const"))
        sem_data = [ctx.enter_context(nc.semaphore(f"sem_d{j}"))
                    for j in range(NDATA_DMA)]
        sem_out = ctx.enter_context(nc.semaphore("sem_out"))
        pe_sem = ctx.enter_context(nc.semaphore("pe_sem"))
        act_sem = ctx.enter_context(nc.semaphore("act_sem"))
        dve_sem = ctx.enter_context(nc.semaphore("dve_sem"))
        block = ctx.enter_context(nc.Block())

        ident = cs[:, C_ID:C_ID + 128]
        b1 = cs[:, C_B1:C_B1 + 1]
        b2 = cs[DT:D1, C_B2:C_B2 + 1]
        obc = cs[0:1, C_OW:C_OW + 1]
        owc = owf[DT:D1, :]
        # logits psum by chunk parity; odd chunks reuse the (long dead)
        # ps_w bank
        ps3 = [ps3_0[0:1, 0:CHUNK], ps_w[0:1, 0:CHUNK]]

        def ps1_ap(c):
            return ps1[c % 2][:, 0:CHUNK]

        def ps2_ap(c):
            return ps2[c % 2][:, 0:CHUNK]

        def h1_ap(s, c):
            off = (s * NCHUNK + c) * CHUNK
            return h1s[:, off:off + CHUNK]

        def h2_ap(s, c):
            # rows DT:D1 so SBUF operands share base partitions with
            # w1t[DT:], ow, and the mm2 psum rows
            off = (s * NCHUNK + c) * CHUNK
            return h2s[DT:D1, off:off + CHUNK]

        def data_dma(eng, j):
            rows = slice(j * TILES_PER_DMA * PTILE,
                         (j + 1) * TILES_PER_DMA * PTILE)
            eng.dma_start(
                ldx[:, rows],
                xc0_d[rows, :].rearrange("(t p) d -> p t d", p=PTILE),
            ).then_inc(sem_data[j], 16)

        @block.sync
        def _(sync):
            # first half of the data rides the ACT engine's HWDGE queue (see
            # block.scalar) so the two queues stream in parallel
            sync.dma_start(cs[:, :], const_d[:, :]).then_inc(sem_const, 16)
            for j in range(NDATA_DMA // 2, NDATA_DMA):
                data_dma(sync, j)
            sync.wait_ge(dve_sem, dve_prod)
            sync.dma_start(out_d[:, :], prod[:, :]).then_inc(sem_out, 16)
            sync.wait_ge(sem_out, 16)

        @block.tensor
        def _(tensor):
            wm = {}

            def twait(sem, val):
                # monotone watermark: skip waits already implied by an
                # earlier wait on the same semaphore
                if wm.get(id(sem), 0) < val:
                    wm[id(sem)] = val
                    tensor.wait_ge(sem, val)

            # the identity lives in the const pack
            twait(sem_const, 16)
            for t in range(NTILE):
                if t % TILES_PER_DMA == 0:
                    twait(sem_data[t // TILES_PER_DMA], 16)
                tsl = slice(t * PTILE, (t + 1) * PTILE)
                nc.tensor.transpose(
                    pst[:, tsl], ldx[:, tsl], ident
                ).then_inc(pe_sem, 1)
            for s in range(kk):
                for c in range(NCHUNK):
                    sl = slice(c * CHUNK, (c + 1) * CHUNK)
                    if s == 0:
                        twait(dve_sem, dve_xct[c])
                    # psum-bank WAR: the previous user of this parity bank
                    # must have been evicted (h1 of chunk c-2, or of the
                    # previous step's chunk c+2)
                    if c >= 2:
                        twait(act_sem, act_h1(s, c - 2))
                    elif s >= 1:
                        twait(act_sem, act_h1(s - 1, c + NCHUNK - 2))
                    # x0-part issues before the h2 wait
                    nc.tensor.matmul(
                        ps1_ap(c), _r(w1t[:, :]), _r(xcT[:, sl]),
                        start=True, stop=(s == 0),
                    ).then_inc(pe_sem, 1)
                    for t in range(s):
                        if t == s - 1:
                            twait(act_sem, act_h2(s - 1, c))
                        nc.tensor.matmul(
                            ps1_ap(c), _r(w1t[DT:D1, :]), _r(h2_ap(t, c)),
                            start=False, stop=(t == s - 1),
                        ).then_inc(pe_sem, 1)
                for c in range(NCHUNK):
                    twait(act_sem, act_h1(s, c))
                    if c >= 2:
                        twait(act_sem, act_h2(s, c - 2))
                    elif s >= 1:
                        twait(act_sem, act_h2(s - 1, c + NCHUNK - 2))
                    nc.tensor.matmul(
                        ps2_ap(c), _r(w2t[:, :]), _r(h1_ap(s, c)),
                        start=True, stop=True,
                    ).then_inc(pe_sem, 1)
            for c in range(NCHUNK):
                sl = slice(c * CHUNK, (c + 1) * CHUNK)
                if kk == 0:
                    twait(dve_sem, dve_xct[c])
                if c >= 2:
                    twait(act_sem, act_sig(c - 2))
                nc.tensor.matmul(
                    ps3[c % 2], _r(owc), _r(xcT[DT:D1, sl]),
                    start=True, stop=(kk == 0),
                ).then_inc(pe_sem, 1)
                for s in range(kk):
                    if s == kk - 1:
                        twait(act_sem, act_h2(kk - 1, c))
                    nc.tensor.matmul(
                        ps3[c % 2], _r(owc), _r(h2_ap(s, c)),
                        start=False, stop=(s == kk - 1),
                    ).then_inc(pe_sem, 1)

        @block.scalar
        def _(scalar):
            # first half of the data DMAs ride this engine's HWDGE queue,
            # in parallel with the SP queue (no waits: issue immediately)
            for j in range(NDATA_DMA // 2):
                data_dma(scalar, j)
            # Prewarm both ACT tables (leaky_relu and sigmoid never share an
            # act-func-set; each load is ~1.3us) immediately at t=0: the
            # input is the framework's preamble-memset const-0.0 cell, so no
            # DMA wait is needed and both loads finish well before the first
            # real eviction.  (Bias reads from `cs` are ordered behind the
            # const DMA transitively through each eviction's PE wait.)
            zcell = nc.const_aps.aps[(mybir.dt.float32, 0.0)][0:1, 0:1]
            nc.scalar.activation(
                warm[0:1, 0:1], zcell, SIGMOID
            ).then_inc(act_sem, 1)
            nc.scalar.activation(
                warm[0:1, 1:2], zcell, LRELU, alpha=0.01
            ).then_inc(act_sem, 1)
            seen = 0
            for s in range(kk):
                for c in range(NCHUNK):
                    if pe_mm1[(s, c)] > seen:
                        seen = pe_mm1[(s, c)]
                        scalar.wait_ge(pe_sem, seen)
                    nc.scalar.activation(
                        _r(h1_ap(s, c)), ps1_ap(c), LRELU,
                        bias=b1, alpha=0.01,
                    ).then_inc(act_sem, 1)
                for c in range(NCHUNK):
                    if pe_mm2[(s, c)] > seen:
                        seen = pe_mm2[(s, c)]
                        scalar.wait_ge(pe_sem, seen)
                    nc.scalar.activation(
                        _r(h2_ap(s, c)), ps2[c % 2][DT:D1, 0:CHUNK], LRELU,
                        bias=b2, alpha=0.01,
                    ).then_inc(act_sem, 1)
            for c in range(NCHUNK):
                sl = slice(c * CHUNK, (c + 1) * CHUNK)
                if pe_mm3[c] > seen:
                    seen = pe_mm3[c]
                    scalar.wait_ge(pe_sem, seen)
                nc.scalar.activation(
                    sig[0:1, sl], ps3[c % 2], SIGMOID, bias=obc
                ).then_inc(act_sem, 1)

        @block.vector
        def _(vector):
            # float32r-rounding passes over the host-pretransposed
            # weights (the verifier requires matmul operands to come from a
            # rounding instruction, which a DMA is not)
            vector.wait_ge(sem_const, 16)
            nc.vector.tensor_copy(
                out=_r(w1t[:, :]), in_=cs[:, C_W1:C_W1 + 128]
            ).then_inc(dve_sem, 1)
            nc.vector.tensor_copy(
                out=_r(w2t[:, :]), in_=cs[:, C_W2:C_W2 + 128]
            ).then_inc(dve_sem, 1)
            nc.vector.tensor_copy(
                out=_r(owc), in_=cs[DT:D1, C_OW:C_OW + 1]
            ).then_inc(dve_sem, 1)
            for c in range(NXCOPY):
                sl = slice(c * 512, (c + 1) * 512)
                vector.wait_ge(pe_sem, (c + 1) * (512 // PTILE))
                nc.vector.tensor_copy(
                    out=_r(xcT[:, sl]), in_=pst[:, sl]
                ).then_inc(dve_sem, 1)
            # per-graph product as two independent half-trees: the first
            # half (graphs of sigmoid chunk 0) runs while chunk 1's logits
            # and sigmoid are still in flight
            GH = G // 2
            dve_val = 3 + NXCOPY
            for hidx in range(2):
                vector.wait_ge(act_sem, act_sig(hidx * (NCHUNK // 2)
                                                + NCHUNK // 2 - 1))

                def gview(tensor, off, length):
                    ap = tensor[0:1, off:off + GH * length]
                    return ap.rearrange("p (g j) -> p g j", g=GH)

                prev_t, prev_off, half = sig, hidx * (R // 2), K // 2
                dst_off = hidx * (R // 2)
                first = True
                while half >= 1:
                    if not first:
                        # DVE completion is not implied by issue order;
                        # chained levels need an explicit completion wait
                        vector.wait_ge(dve_sem, dve_val)
                    pv = gview(prev_t, prev_off, 2 * half)
                    if half == 1:
                        dst = prod[0:1, hidx * GH:(hidx + 1) * GH].rearrange(
                            "p (g j) -> p g j", g=GH)
                    else:
                        dst = gview(ptree, dst_off, half)
                    nc.vector.tensor_tensor(
                        dst, pv[:, :, 0:half], pv[:, :, half:2 * half],
                        mybir.AluOpType.mult,
                    ).then_inc(dve_sem, 1)
                    dve_val += 1
                    first = False
                    prev_t, prev_off = ptree, dst_off
                    dst_off += GH * half
                    half //= 2

    return nc


def _pack_consts(w1, b1, w2, b2, ow, ob):
    cp = np.zeros((128, CW), np.float32)
    cp[:, C_ID:C_ID + 128] = np.eye(128, dtype=np.float32)
    # weights pre-transposed on host (raw Bass needs no on-device
    # const-sem staging, and PE transposes here would delay the data tiles)
    cp[:, C_W1:C_W1 + 128] = w1.T
    cp[:, C_W2 + DT:C_W2 + D1] = w2.T
    cp[:, C_B1] = b1
    cp[DT:D1, C_B2] = b2
    cp[DT:D1, C_OW] = ow.reshape(D2)
    cp[0, C_OW] = ob.reshape(())
    return cp


def _make_in_maps(towers, x, w1, b1, w2, b2, ow, ob):
    towers = np.asarray(towers, np.float32)
    x = np.asarray(x, np.float32)
    cpack = _pack_consts(
        np.asarray(w1, np.float32), np.asarray(b1, np.float32),
        np.asarray(w2, np.float32), np.asarray(b2, np.float32),
        np.asarray(ow, np.float32), np.asarray(ob, np.float32),
    )
    xc0 = np.concatenate(
        [towers.reshape(N * K, DT), x.reshape(N * K, D2)], axis=1
    )
    in_maps = []
    for i in range(N_CORES):
        sl = slice(i * R, (i + 1) * R)
        in_maps.append({"cpack": cpack, "xc0": np.ascontiguousarray(xc0[sl])})
    return in_maps


def kernel(towers, x, w1, b1, w2, b2, aw1, ab1, aw2, ab2, ow, ob, k):
    global LAST_RESULT
    kk = int(k)

    if kk not in _PROGRAM_CACHE:
        _PROGRAM_CACHE[kk] = _build_program(kk)
    nc = _PROGRAM_CACHE[kk]

    in_maps = _make_in_maps(towers, x, w1, b1, w2, b2, ow, ob)
    res = run_bass_kernel_spmd(nc, in_maps, list(range(N_CORES)))
    LAST_RESULT = res
    out = np.concatenate(
        [np.asarray(res.results[i]["out"]).reshape(G) for i in range(N_CORES)]
    )
    return out.astype(np.float32)



# revision 8
# speedup vs baseline: 1.3598x; 1.3598x over previous
"""Trainium2 Bass kernel for nn_FCGAT (fully-connected GAT variant).

Mathematical simplifications (exact, same as the v1 kernel):

1. ``einsum('nkj,nkd->nkd', softmax(aa,2), h) == h`` (softmax rows sum to 1),
   so the whole attention block is dead code and the model reduces to::

       h_s = lrelu(lrelu([towers | x_s] @ w1.T + b1) @ w2.T + b2)
       x_{s+1} = h_s + x_s
       out_n = prod_k sigmoid(x_K @ ow[0] + ob[0])

2. Residuals are distributed through the linear maps as PSUM accumulation
   groups: ``x_s = x_0 + sum_{t<s} h_t`` so no elementwise adds exist.

v2 speedups over the 15.6us v1 kernel (CoreSim cost model):

* bf16 everywhere on the data path (inputs rounded on host, weights
  pre-rounded, hidden activations stored bf16; PSUM stays f32).  Halves the
  input DMA and makes every matmul 1 PE cycle/row.  Measured end-to-end
  numeric error vs a float64 oracle: 5.5e-3 (budget 2e-2).
* host-side transpose of the input to feature-major [D1, rows] removes the
  8 on-device PE transposes + DVE rounding copies of v1 entirely.
* 3 input DMAs ride 3 different queues (SP / ACT HWDGE + Pool SWDGE) which
  the cost model executes fully in parallel: all input sems fire at ~2.4us
  (the fixed DMA latency floor).
* Prelu (== leaky_relu with alpha) replaces Lrelu: parametric_relu lives in
  the SAME activation-table set as Sigmoid, so ONE table load (prewarmed at
  t=0 under the DMA shadow) covers the whole kernel.  v1 paid 2 prewarm
  loads plus a 1.3us mid-kernel reload for the final sigmoid.
* the logits matmul is restructured: instead of [1, R] output rows on one
  partition (v1: 612ns+ sigmoid, then a 1.5us DVE product tree), each
  128-row tile becomes a *stationary* operand against the moving ow column,
  yielding logits [128, 8] across partitions.  Sigmoid is then a 192ns ACT
  op; one PE transpose + two DVE cumprod scans (tensor_tensor_scan, op0=mult)
  produce the 16 per-graph products directly.

Sharding: data-parallel over N=128 -> 16 graphs (1024 rows) per core across
8 NeuronCores; weights replicated.

Raw Bass (explicit engine blocks + standalone single-condition waits), same
rationale as v1: the walrus build allows only one sync-wait per instruction.
"""

from contextlib import ExitStack

import ml_dtypes
import numpy as np

import concourse.bass as bass
import concourse.mybir as mybir
from concourse.bass_utils import run_bass_kernel_spmd

N_CORES = 8
N, K, DT, D2 = 128, 64, 64, 64
D1 = DT + D2                # 128: [towers | x] feature dim
G = N // N_CORES            # 16 graphs per core
R = G * K                   # 1024 rows per core
CHUNK = 512                 # psum-bank / moving-operand chunk
NCHUNK = R // CHUNK         # 2
TILE = 128                  # logits stationary tile (rows)
NT = R // TILE              # 8 logits tiles -> psum [128, NT]

# packed constants layout (f32 columns of a [128, CW] array; bf16 payloads
# are stored as raw bytes and bitcast on device)
C_ID = 0                    # 0:128    identity (f32, final transpose)
C_W1B = 128                 # 128:192  w1.T as bf16 [128, 128]
C_W2B = 192                 # 192:224  w2.T as bf16 [128, 64]
C_OWB = 224                 # col 224, rows 64:128: ow[0] as bf16 (low half)
C_B1 = 225                  # b1 f32 column
C_B2 = 226                  # b2 f32 column, rows 64:128
C_OB = 227                  # ob f32 replicated on all 128 rows
C_Z = 228                   # 228:292  zeros (f32; dummy data1 for the scans)
CW = 292

_F32 = mybir.dt.float32
_BF16 = mybir.dt.bfloat16

LAST_RESULT = None
_PROGRAM_CACHE = {}


def _build_program(kk: int, act_fn=None, sig_fn=None) -> bass.Bass:
    """act_fn/sig_fn overrides exist for CoreSim exec-mode validation
    (CoreSim implements Relu/Sigmoid but not Prelu)."""
    PRELU = act_fn or mybir.ActivationFunctionType.Prelu
    SIGMOID = sig_fn or mybir.ActivationFunctionType.Sigmoid

    nc = bass.Bass()
    const_d = nc.declare_dram_parameter("cpack", [128, CW], _F32,
                                        isOutput=False)
    xd = nc.declare_dram_parameter("xc0T", [128, R], _BF16, isOutput=False)
    out_d = nc.declare_dram_parameter("out", [NT, 2], _F32, isOutput=True)

    with ExitStack() as ctx:
        cs = ctx.enter_context(nc.sbuf_tensor([128, CW], _F32))
        xcT = ctx.enter_context(nc.sbuf_tensor([128, R], _BF16))
        h1s = ctx.enter_context(nc.sbuf_tensor([128, R], _BF16))
        h2s = [ctx.enter_context(nc.sbuf_tensor(f"h2s_{s}", [128, R], _BF16))
               for s in range(kk)]
        sig = ctx.enter_context(nc.sbuf_tensor([128, NT], _F32))
        prod = ctx.enter_context(nc.sbuf_tensor([NT, TILE], _F32))
        warm = ctx.enter_context(nc.sbuf_tensor([1, 1], _F32))
        # full-bank psum allocations (avoid same-bank PE-write/engine-read)
        ps1 = [ctx.enter_context(nc.psum_tensor(f"ps1_{c}", [128, 512], _F32))
               for c in range(NCHUNK)]
        ps2 = [ctx.enter_context(nc.psum_tensor(f"ps2_{c}", [128, 512], _F32))
               for c in range(NCHUNK)]
        ps3 = ctx.enter_context(nc.psum_tensor([128, 512], _F32))
        pst = ctx.enter_context(nc.psum_tensor([128, 512], _F32))
        sem_const = ctx.enter_context(nc.semaphore("sem_const"))
        sem_d = [ctx.enter_context(nc.semaphore(f"sem_d{c}"))
                 for c in range(NCHUNK)]
        sem_out = ctx.enter_context(nc.semaphore("sem_out"))
        pe_sem = ctx.enter_context(nc.semaphore("pe_sem"))
        act_sem = ctx.enter_context(nc.semaphore("act_sem"))
        dve_sem = ctx.enter_context(nc.semaphore("dve_sem"))
        block = ctx.enter_context(nc.Block())

        ident = cs[:, C_ID:C_ID + 128]
        w1t = cs[:, C_W1B:C_W1B + 64].bitcast(_BF16)        # [128, 128]
        w2t = cs[:, C_W2B:C_W2B + 32].bitcast(_BF16)        # [128, 64]
        owc = cs[DT:D1, C_OWB:C_OWB + 1].bitcast(_BF16)[:, 0:1]  # [64, 1]
        b1 = cs[:, C_B1:C_B1 + 1]
        b2 = cs[DT:D1, C_B2:C_B2 + 1]
        obc = cs[:, C_OB:C_OB + 1]
        zrow = cs[0:NT, C_Z:C_Z + 64]                       # [8, 64] zeros

        def csl(c):
            return slice(c * CHUNK, (c + 1) * CHUNK)

        # ---- PE instruction numbering (emitted below in this order) ----
        pe_n = 0
        pe_mm1 = {}   # (s, c) -> pe value after the mm1 group for (s, c)
        pe_mm2 = {}   # (s, c)
        pe_mm3_last = 0
        pe_transpose = 0
        act_n = 1     # warm == 1
        act_h1 = {}
        act_h2 = {}
        for s in range(kk):
            for c in range(NCHUNK):
                act_h1[(s, c)] = act_n + 1
                act_n += 1
            for c in range(NCHUNK):
                act_h2[(s, c)] = act_n + 1
                act_n += 1
        act_sig = act_n + 1

        @block.gpsimd
        def _(gpsimd):
            gpsimd.dma_start(cs[:, :], const_d[:, :]).then_inc(sem_const, 16)

        @block.sync
        def _(sync):
            sync.dma_start(xcT[:, csl(0)], xd[:, csl(0)]).then_inc(
                sem_d[0], 16)
            sync.wait_ge(dve_sem, 2)
            with nc.allow_non_contiguous_dma(
                    reason="16 scalars (graph products), 16 descriptors"):
                sync.dma_start(
                    out_d[:, :],
                    prod[:, :].rearrange("p (a b) -> p a b", a=2)[:, :, 63:64],
                ).then_inc(sem_out, 16)
            sync.wait_ge(sem_out, 16)

        @block.tensor
        def _(tensor):
            nonlocal pe_mm3_last, pe_transpose
            wm = {}

            def twait(sem, val):
                if wm.get(id(sem), 0) < val:
                    wm[id(sem)] = val
                    tensor.wait_ge(sem, val)

            pe = 0

            def inc(instr):
                nonlocal pe
                pe += 1
                instr.then_inc(pe_sem, 1)

            def mm3_column(t):
                # one complete accumulation group per psum column: the PSUM
                # start bit zeroes the whole 2KB bank region, so groups in a
                # bank must be strictly sequential, not interleaved
                tsl = slice(t * TILE, (t + 1) * TILE)
                terms = [xcT[DT:D1, tsl]] + [
                    h2s[s][DT:D1, tsl] for s in range(kk)]
                for i, lhsT in enumerate(terms):
                    inc(nc.tensor.matmul(
                        ps3[:, t:t + 1], lhsT, owc,
                        start=(i == 0), stop=(i == len(terms) - 1),
                    ))

            # step-0 mm1
            twait(sem_const, 16)
            for c in range(NCHUNK):
                twait(sem_d[c], 16)
                inc(nc.tensor.matmul(
                    ps1[c][:, :], w1t, xcT[:, csl(c)],
                    start=True, stop=True))
                pe_mm1[(0, c)] = pe
            for s in range(kk):
                last_step = (s == kk - 1)
                for c in range(NCHUNK):
                    twait(act_sem, act_h1[(s, c)])
                    inc(nc.tensor.matmul(
                        ps2[c][DT:D1, :], w2t, h1s[:, csl(c)],
                        start=True, stop=True))
                    pe_mm2[(s, c)] = pe
                    if not last_step:
                        # next-step mm1 x0 part; ps1[c] WAR cleared by the
                        # act_h1 wait just above
                        inc(nc.tensor.matmul(
                            ps1[c][:, :], w1t, xcT[:, csl(c)],
                            start=True, stop=False))
                if not last_step:
                    for c in range(NCHUNK):
                        for t in range(s + 1):
                            if t == s:
                                twait(act_sem, act_h2[(s, c)])
                            inc(nc.tensor.matmul(
                                ps1[c][:, :], w1t[DT:D1, :],
                                h2s[t][DT:D1, csl(c)],
                                start=False, stop=(t == s)))
                        pe_mm1[(s + 1, c)] = pe
            for c in range(NCHUNK):
                if kk > 0:
                    twait(act_sem, act_h2[(kk - 1, c)])
                for t in range(c * (NT // NCHUNK), (c + 1) * (NT // NCHUNK)):
                    mm3_column(t)
            pe_mm3_last = pe
            twait(act_sem, act_sig)
            inc(nc.tensor.transpose(pst[0:NT, 0:128], sig[:, 0:NT], ident))
            pe_transpose = pe

        @block.scalar
        def _(scalar):
            scalar.dma_start(xcT[:, csl(1)], xd[:, csl(1)]).then_inc(
                sem_d[1], 16)
            zcell = nc.const_aps.aps[(mybir.dt.float32, 0.0)][0:1, 0:1]
            nc.scalar.activation(warm[0:1, 0:1], zcell, SIGMOID).then_inc(
                act_sem, 1)
            seen = 0

            def swait(val):
                nonlocal seen
                if val > seen:
                    seen = val
                    scalar.wait_ge(pe_sem, val)

            for s in range(kk):
                for c in range(NCHUNK):
                    swait(pe_mm1[(s, c)])
                    nc.scalar.activation(
                        h1s[:, csl(c)], ps1[c][:, :], PRELU,
                        bias=b1, alpha=0.01,
                    ).then_inc(act_sem, 1)
                for c in range(NCHUNK):
                    swait(pe_mm2[(s, c)])
                    nc.scalar.activation(
                        h2s[s][DT:D1, csl(c)], ps2[c][DT:D1, :], PRELU,
                        bias=b2, alpha=0.01,
                    ).then_inc(act_sem, 1)
            swait(pe_mm3_last)
            nc.scalar.activation(
                sig[:, 0:NT], ps3[:, 0:NT], SIGMOID, bias=obc,
            ).then_inc(act_sem, 1)

        @block.vector
        def _(vector):
            vector.wait_ge(pe_sem, pe_transpose)
            for half in range(2):
                hsl = slice(half * 64, half * 64 + 64)
                nc.vector.tensor_tensor_scan(
                    prod[0:NT, hsl], pst[0:NT, hsl], zrow,
                    initial=1.0,
                    op0=mybir.AluOpType.mult, op1=mybir.AluOpType.bypass,
                ).then_inc(dve_sem, 1)

    return nc


def _pack_consts(w1, b1, w2, b2, ow, ob):
    cp = np.zeros((128, CW), np.float32)
    cp[:, C_ID:C_ID + 128] = np.eye(128, dtype=np.float32)
    bv = cp.view(np.uint8)
    bf = ml_dtypes.bfloat16
    bv[:, C_W1B * 4:C_W1B * 4 + 256] = np.ascontiguousarray(
        w1.T.astype(bf)).view(np.uint8)
    bv[:, C_W2B * 4:C_W2B * 4 + 128] = np.ascontiguousarray(
        w2.T.astype(bf)).view(np.uint8)
    bv[DT:D1, C_OWB * 4:C_OWB * 4 + 2] = np.ascontiguousarray(
        ow.reshape(D2, 1).astype(bf)).view(np.uint8)
    cp[:, C_B1] = b1
    cp[DT:D1, C_B2] = b2
    cp[:, C_OB] = np.float32(ob.reshape(())[()])
    return cp


def _make_in_maps(towers, x, w1, b1, w2, b2, ow, ob):
    towers = np.asarray(towers, np.float32)
    x = np.asarray(x, np.float32)
    cpack = _pack_consts(
        np.asarray(w1, np.float32), np.asarray(b1, np.float32),
        np.asarray(w2, np.float32), np.asarray(b2, np.float32),
        np.asarray(ow, np.float32), np.asarray(ob, np.float32),
    )
    xcT = np.concatenate(
        [towers.reshape(N * K, DT), x.reshape(N * K, D2)], axis=1
    ).T.astype(ml_dtypes.bfloat16)          # [128, N*K]
    in_maps = []
    for i in range(N_CORES):
        sl = slice(i * R, (i + 1) * R)
        in_maps.append({
            "cpack": cpack,
            "xc0T": np.ascontiguousarray(xcT[:, sl]),
        })
    return in_maps


def kernel(towers, x, w1, b1, w2, b2, aw1, ab1, aw2, ab2, ow, ob, k):
    global LAST_RESULT
    kk = int(k)

    if kk not in _PROGRAM_CACHE:
        _PROGRAM_CACHE[kk] = _build_program(kk)
    nc = _PROGRAM_CACHE[kk]

    in_maps = _make_in_maps(towers, x, w1, b1, w2, b2, ow, ob)
    res = run_bass_kernel_spmd(nc, in_maps, list(range(N_CORES)))
    LAST_RESULT = res
    # out[t, h] = product of graph 2t+h of that core
    out = np.concatenate([
        np.asarray(res.results[i]["out"]).reshape(G) for i in range(N_CORES)
    ])
    return out.astype(np.float32)


# revision 70
# speedup vs baseline: 1.3888x; 1.0213x over previous
"""Trainium2 Bass kernel for nn_FCGAT (fully-connected GAT variant).

Mathematical simplifications (exact, same as the v1 kernel):

1. ``einsum('nkj,nkd->nkd', softmax(aa,2), h) == h`` (softmax rows sum to 1),
   so the whole attention block is dead code and the model reduces to::

       h_s = lrelu(lrelu([towers | x_s] @ w1.T + b1) @ w2.T + b2)
       x_{s+1} = h_s + x_s
       out_n = prod_k sigmoid(x_K @ ow[0] + ob[0])

2. Residuals are distributed through the linear maps as PSUM accumulation
   groups: ``x_s = x_0 + sum_{t<s} h_t`` so no elementwise adds exist.

v2 speedups over the 15.6us v1 kernel (CoreSim cost model):

* bf16 everywhere on the data path (inputs rounded on host, weights
  pre-rounded, hidden activations stored bf16; PSUM stays f32).  Halves the
  input DMA and makes every matmul 1 PE cycle/row.  Measured end-to-end
  numeric error vs a float64 oracle: 5.5e-3 (budget 2e-2).
* host-side transpose of the input to feature-major [D1, rows] removes the
  8 on-device PE transposes + DVE rounding copies of v1 entirely.
* 3 input DMAs ride 3 different queues (SP / ACT HWDGE + Pool SWDGE) which
  the cost model executes fully in parallel: all input sems fire at ~2.4us
  (the fixed DMA latency floor).
* Prelu (== leaky_relu with alpha) replaces Lrelu: parametric_relu lives in
  the SAME activation-table set as Sigmoid, so ONE table load (prewarmed at
  t=0 under the DMA shadow) covers the whole kernel.  v1 paid 2 prewarm
  loads plus a 1.3us mid-kernel reload for the final sigmoid.
* the logits matmul is restructured: instead of [1, R] output rows on one
  partition (v1: 612ns+ sigmoid, then a 1.5us DVE product tree), each
  128-row tile becomes a *stationary* operand against the moving ow column,
  yielding logits [128, 8] across partitions.  Sigmoid is then a 192ns ACT
  op; one PE transpose + two DVE cumprod scans (tensor_tensor_scan, op0=mult)
  produce the 16 per-graph products directly.

Sharding: data-parallel over N=128 -> 16 graphs (1024 rows) per core across
8 NeuronCores; weights replicated.

Raw Bass (explicit engine blocks + standalone single-condition waits), same
rationale as v1: the walrus build allows only one sync-wait per instruction.
"""

from contextlib import ExitStack

import ml_dtypes
import numpy as np

import concourse.bass as bass
import concourse.mybir as mybir
from concourse.bass_utils import run_bass_kernel_spmd

N_CORES = 8
N, K, DT, D2 = 128, 64, 64, 64
D1 = DT + D2                # 128: [towers | x] feature dim
G = N // N_CORES            # 16 graphs per core
R = G * K                   # 1024 rows per core
CHUNK = 512                 # psum-bank / moving-operand chunk
NCHUNK = R // CHUNK         # 2
TILE = 128                  # logits stationary tile (rows)
NT = R // TILE              # 8 logits tiles -> psum [128, NT]

# packed constants layout (f32 columns of a [128, CW] array; bf16 payloads
# are stored as raw bytes and bitcast on device)
C_ID = 0                    # 0:128    identity (f32, final transpose)
C_W1B = 128                 # 128:192  w1.T as bf16 [128, 128]
C_W2B = 192                 # 192:224  w2.T as bf16 [128, 64]
C_OWB = 224                 # col 224, rows 64:128: ow[0] as bf16 (low half)
C_B1 = 225                  # b1 f32 column
C_B2 = 226                  # b2 f32 column, rows 64:128
C_OB = 227                  # ob f32 replicated on all 128 rows
C_Z = 228                   # 228:292  zeros (f32; scan dummy + output zeroing)
CW = 292

_F32 = mybir.dt.float32
_BF16 = mybir.dt.bfloat16

# FAST_OUT (SWDGE scatter-add + trigger output path) cuts ~2.1us off the
# CoreSim tail but the gpsimd dma_scatter_add Q7 kernel crashes the device
# on this terminal's firmware (verified with isolated probes in both
# prepare/trigger and immediate modes; a plain load_library works).  Keep
# the implementation but ship with the plain HWDGE output DMA.
FAST_OUT = False
DVE_COLS = 0

LAST_RESULT = None
_PROGRAM_CACHE = {}


def _build_program(kk: int, act_fn=None, sig_fn=None, out_sync=2,
                   fast_out=FAST_OUT, dve_cols=DVE_COLS,
                   dve_h1_cols=0) -> bass.Bass:
    """act_fn/sig_fn overrides exist for CoreSim exec-mode validation
    (CoreSim implements Relu/Sigmoid but not Prelu).

    out_sync: 2 = output DMA incs sem_out and SP waits on it (fully safe);
    1 = inc only, no wait; 0 = no completion sem at all.

    dve_cols: offload the last dve_cols columns of every h2 chunk from the
    saturated ACT engine to the otherwise-idle DVE as
    max(x + b2, 0.01*(x + b2)) (two DVE ops).  Must be a multiple of 64 so
    the per-graph logits slices don't straddle the ACT/DVE boundary.

    fast_out: write the 16 products with a SWDGE scatter-add whose
    descriptors are prepared at t~1us and fired by trigger_dma at the end.
    The trigger path skips the 625ns HWDGE + 650ns DGE delay of a regular
    DMA, cutting ~1.2us off the tail.  Scatter-add ACCUMULATES into HBM, so
    the output buffer is zeroed by an early DMA (completion-sem ordered
    before the trigger).  fast_out=False falls back to a plain SP-queue
    DMA."""
    PRELU = act_fn or mybir.ActivationFunctionType.Prelu
    SIGMOID = sig_fn or mybir.ActivationFunctionType.Sigmoid
    DD = dve_cols                   # DVE columns per h2 chunk
    AA = CHUNK - DD                 # ACT columns per h2 chunk
    HH = dve_h1_cols                # DVE columns per h1 chunk
    AH = CHUNK - HH                 # ACT columns per h1 chunk
    assert DD % 64 == 0 and 0 <= DD < CHUNK
    assert HH % 64 == 0 and 0 <= HH < CHUNK

    nc = bass.Bass()
    const_d = nc.declare_dram_parameter("cpack", [128, CW], _F32,
                                        isOutput=False)
    xd = nc.declare_dram_parameter("xc0T", [128, R], _BF16, isOutput=False)
    out_d = nc.declare_dram_parameter("out", [G, 64 if fast_out else 1],
                                      _F32, isOutput=True)
    idx_d = (nc.declare_dram_parameter("idxs", [128, 8], mybir.dt.int16,
                                       isOutput=False) if fast_out else None)

    with ExitStack() as ctx:
        cs = ctx.enter_context(nc.sbuf_tensor([128, CW], _F32))
        idxs_sb = ctx.enter_context(
            nc.sbuf_tensor("idxs_sb", [128, 8], mybir.dt.int16))
        tmp = ctx.enter_context(
            nc.sbuf_tensor("tmp", [128, max(1, 2 * kk * max(DD, 1))], _BF16))
        tmph = ctx.enter_context(
            nc.sbuf_tensor("tmph", [128, max(1, 2 * kk * max(HH, 1))],
                           _BF16))
        xcT = ctx.enter_context(nc.sbuf_tensor([128, R], _BF16))
        h1s = ctx.enter_context(nc.sbuf_tensor([128, R], _BF16))
        h2s = [ctx.enter_context(nc.sbuf_tensor(f"h2s_{s}", [128, R], _BF16))
               for s in range(kk)]
        sig = ctx.enter_context(nc.sbuf_tensor([128, G], _F32))
        prod = ctx.enter_context(nc.sbuf_tensor([128, K], _F32))
        warm = ctx.enter_context(nc.sbuf_tensor([1, 1], _F32))
        # full-bank psum allocations (avoid same-bank PE-write/engine-read)
        ps1 = [ctx.enter_context(nc.psum_tensor(f"ps1_{c}", [128, 512], _F32))
               for c in range(NCHUNK)]
        ps2 = [ctx.enter_context(nc.psum_tensor(f"ps2_{c}", [128, 512], _F32))
               for c in range(NCHUNK)]
        ps3 = ctx.enter_context(nc.psum_tensor([128, 512], _F32))
        pst = ctx.enter_context(nc.psum_tensor([128, 512], _F32))
        sem_const = ctx.enter_context(nc.semaphore("sem_const"))
        sem_d = [ctx.enter_context(nc.semaphore(f"sem_d{c}"))
                 for c in range(NCHUNK)]
        sem_out = ctx.enter_context(nc.semaphore("sem_out"))
        sem_zero = ctx.enter_context(nc.semaphore("sem_zero"))
        sem_prep = ctx.enter_context(nc.semaphore("sem_prep"))
        sem_idx = ctx.enter_context(nc.semaphore("sem_idx"))
        pe_sem = ctx.enter_context(nc.semaphore("pe_sem"))
        act_sem = ctx.enter_context(nc.semaphore("act_sem"))
        dve_sem = ctx.enter_context(nc.semaphore("dve_sem"))
        block = ctx.enter_context(nc.Block())

        ident = cs[:, C_ID:C_ID + 128]
        w1t = cs[:, C_W1B:C_W1B + 64].bitcast(_BF16)        # [128, 128]
        w2t = cs[:, C_W2B:C_W2B + 32].bitcast(_BF16)        # [128, 64]
        owc = cs[DT:D1, C_OWB:C_OWB + 1].bitcast(_BF16)[:, 0:1]  # [64, 1]
        b1 = cs[:, C_B1:C_B1 + 1]
        b2 = cs[DT:D1, C_B2:C_B2 + 1]
        obc = cs[:, C_OB:C_OB + 1]
        zrow = cs[0:G, C_Z:C_Z + K]                         # [16, 64] zeros

        def csl(c):
            return slice(c * CHUNK, (c + 1) * CHUNK)

        # ---- PE instruction numbering (emitted below in this order) ----
        pe_n = 0
        pe_mm1 = {}   # (s, c) -> pe value after the mm1 group for (s, c)
        pe_mm2 = {}   # (s, c)
        pe_mm3_last = 0
        pe_transpose = 0
        act_n = 1     # warm == 1
        act_h1 = {}
        act_h2 = {}
        for s in range(kk):
            for c in range(NCHUNK):
                act_h1[(s, c)] = act_n + 1
                act_n += 1
            for c in range(NCHUNK):
                act_h2[(s, c)] = act_n + 1
                act_n += 1
        act_sig = act_n + 1
        # DVE numbering: memset (fast_out), then per step the h1-offload
        # pairs then the h2-offload pairs, then the cumprod scan
        dve_n = 1 if fast_out else 0
        dve_h1b = {}
        dve_h2b = {}
        for s in range(kk):
            for c in range(NCHUNK):
                if HH:
                    dve_n += 2
                    dve_h1b[(s, c)] = dve_n
            for c in range(NCHUNK):
                if DD:
                    dve_n += 2
                    dve_h2b[(s, c)] = dve_n
        dve_scan = dve_n + 1

        @block.sync
        def _(sync):
            sync.dma_start(xcT[:, csl(0)], xd[:, csl(0)]).then_inc(
                sem_d[0], 16)
            if fast_out:
                sync.dma_start(idxs_sb[:, :], idx_d[:, :]).then_inc(
                    sem_idx, 16)
            if not fast_out:
                sync.wait_ge(dve_sem, dve_scan)
                dma = sync.dma_start(out_d[:, :], prod[0:G, K - 1:K])
                if out_sync >= 1:
                    dma.then_inc(sem_out, 16)
            if out_sync >= 2:
                sync.wait_ge(sem_out, 16)

        @block.tensor
        def _(tensor):
            nonlocal pe_mm3_last, pe_transpose
            wm = {}

            def twait(sem, val):
                if wm.get(id(sem), 0) < val:
                    wm[id(sem)] = val
                    tensor.wait_ge(sem, val)

            pe = 0

            def inc(instr):
                nonlocal pe
                pe += 1
                instr.then_inc(pe_sem, 1)

            def mm3_column(g):
                # one complete accumulation group per psum column (one GRAPH
                # per column: 64 rows -> 64 output partitions): the PSUM
                # start bit zeroes the whole 2KB bank region, so groups in a
                # bank must be strictly sequential, not interleaved
                gsl = slice(g * K, (g + 1) * K)
                terms = [xcT[DT:D1, gsl]] + [
                    h2s[s][DT:D1, gsl] for s in range(kk)]
                for i, lhsT in enumerate(terms):
                    inc(nc.tensor.matmul(
                        ps3[0:K, g:g + 1], lhsT, owc,
                        start=(i == 0), stop=(i == len(terms) - 1),
                    ))

            # step-0 mm1
            twait(sem_const, 16)
            for c in range(NCHUNK):
                twait(sem_d[c], 16)
                inc(nc.tensor.matmul(
                    ps1[c][:, :], w1t, xcT[:, csl(c)],
                    start=True, stop=True))
                pe_mm1[(0, c)] = pe
            for s in range(kk):
                last_step = (s == kk - 1)
                for c in range(NCHUNK):
                    twait(act_sem, act_h1[(s, c)])
                    if HH:
                        twait(dve_sem, dve_h1b[(s, c)])
                    if s >= 1 and DD:
                        # ps2[c] WAR vs previous step's DVE h2 reader
                        twait(dve_sem, dve_h2b[(s - 1, c)])
                    inc(nc.tensor.matmul(
                        ps2[c][DT:D1, :], w2t, h1s[:, csl(c)],
                        start=True, stop=True))
                    pe_mm2[(s, c)] = pe
                    if not last_step:
                        # next-step mm1 x0 part; ps1[c] WAR cleared by the
                        # act_h1 wait just above
                        inc(nc.tensor.matmul(
                            ps1[c][:, :], w1t, xcT[:, csl(c)],
                            start=True, stop=False))
                if not last_step:
                    for c in range(NCHUNK):
                        lo = c * CHUNK
                        for t in range(s):
                            inc(nc.tensor.matmul(
                                ps1[c][:, :], w1t[DT:D1, :],
                                h2s[t][DT:D1, csl(c)],
                                start=False, stop=False))
                        # term t == s arrives in two pieces when the h2
                        # activation is split across ACT and DVE
                        twait(act_sem, act_h2[(s, c)])
                        inc(nc.tensor.matmul(
                            ps1[c][:, 0:AA], w1t[DT:D1, :],
                            h2s[s][DT:D1, lo:lo + AA],
                            start=False, stop=(DD == 0)))
                        if DD:
                            twait(dve_sem, dve_h2b[(s, c)])
                            inc(nc.tensor.matmul(
                                ps1[c][:, AA:CHUNK], w1t[DT:D1, :],
                                h2s[s][DT:D1, lo + AA:lo + CHUNK],
                                start=False, stop=True))
                        pe_mm1[(s + 1, c)] = pe
            for c in range(NCHUNK):
                if kk > 0:
                    twait(act_sem, act_h2[(kk - 1, c)])
                    if DD:
                        twait(dve_sem, dve_h2b[(kk - 1, c)])
                for g in range(c * (G // NCHUNK), (c + 1) * (G // NCHUNK)):
                    mm3_column(g)
            pe_mm3_last = pe
            twait(act_sem, act_sig)
            inc(nc.tensor.transpose(
                pst[0:G, 0:K], sig[0:K, 0:G], ident[0:K, 0:K]))
            pe_transpose = pe

        @block.scalar
        def _(scalar):
            scalar.dma_start(xcT[:, csl(1)], xd[:, csl(1)]).then_inc(
                sem_d[1], 16)
            zcell = nc.const_aps.aps[(mybir.dt.float32, 0.0)][0:1, 0:1]
            nc.scalar.activation(warm[0:1, 0:1], zcell, SIGMOID).then_inc(
                act_sem, 1)
            seen = 0

            def swait(val):
                nonlocal seen
                if val > seen:
                    seen = val
                    scalar.wait_ge(pe_sem, val)

            for s in range(kk):
                for c in range(NCHUNK):
                    lo = c * CHUNK
                    swait(pe_mm1[(s, c)])
                    nc.scalar.activation(
                        h1s[:, lo:lo + AH], ps1[c][:, 0:AH], PRELU,
                        bias=b1, alpha=0.01,
                    ).then_inc(act_sem, 1)
                for c in range(NCHUNK):
                    lo = c * CHUNK
                    swait(pe_mm2[(s, c)])
                    nc.scalar.activation(
                        h2s[s][DT:D1, lo:lo + AA], ps2[c][DT:D1, 0:AA],
                        PRELU, bias=b2, alpha=0.01,
                    ).then_inc(act_sem, 1)
            swait(pe_mm3_last)
            nc.scalar.activation(
                sig[0:K, 0:G], ps3[0:K, 0:G], SIGMOID, bias=obc[0:K, :],
            ).then_inc(act_sem, 1)

        @block.vector
        def _(vector):
            if fast_out:
                # the scatter-add src AP spans all 128 partitions; only
                # 0:G hold real products, the rest must be initialized
                # (they map to ignored negative indices).  Engines need
                # quadrant-aligned base partitions, so clear the full
                # column up front and let the scan overwrite rows 0:G.
                nc.vector.memset(prod[:, K - 1:K], 0.0).then_inc(
                    dve_sem, 1)
            # lrelu offload pairs: out = max(x + b, 0.01*(x + b))
            for s in range(kk):
                for c in range(NCHUNK):
                    if not HH:
                        continue
                    lo = c * CHUNK
                    tsl = slice((s * NCHUNK + c) * HH,
                                (s * NCHUNK + c + 1) * HH)
                    vector.wait_ge(pe_sem, pe_mm1[(s, c)])
                    nc.vector.tensor_scalar(
                        out=tmph[:, tsl], in0=ps1[c][:, AH:CHUNK],
                        scalar1=b1, scalar2=0.01,
                        op0=mybir.AluOpType.add, op1=mybir.AluOpType.mult,
                    ).then_inc(dve_sem, 1)
                    vector.wait_ge(dve_sem, dve_h1b[(s, c)] - 1)
                    nc.vector.scalar_tensor_tensor(
                        out=h1s[:, lo + AH:lo + CHUNK],
                        in0=ps1[c][:, AH:CHUNK], scalar=b1,
                        in1=tmph[:, tsl],
                        op0=mybir.AluOpType.add, op1=mybir.AluOpType.max,
                    ).then_inc(dve_sem, 1)
                for c in range(NCHUNK):
                    if not DD:
                        continue
                    lo = c * CHUNK
                    tsl = slice((s * NCHUNK + c) * DD,
                                (s * NCHUNK + c + 1) * DD)
                    vector.wait_ge(pe_sem, pe_mm2[(s, c)])
                    nc.vector.tensor_scalar(
                        out=tmp[DT:D1, tsl], in0=ps2[c][DT:D1, AA:CHUNK],
                        scalar1=b2, scalar2=0.01,
                        op0=mybir.AluOpType.add, op1=mybir.AluOpType.mult,
                    ).then_inc(dve_sem, 1)
                    # explicit completion wait: DVE RAW on tmp
                    vector.wait_ge(dve_sem, dve_h2b[(s, c)] - 1)
                    nc.vector.scalar_tensor_tensor(
                        out=h2s[s][DT:D1, lo + AA:lo + CHUNK],
                        in0=ps2[c][DT:D1, AA:CHUNK], scalar=b2,
                        in1=tmp[DT:D1, tsl],
                        op0=mybir.AluOpType.add, op1=mybir.AluOpType.max,
                    ).then_inc(dve_sem, 1)
            # explicit ordering: the scan below rewrites rows 0:G of the
            # memset column
            vector.wait_ge(dve_sem, dve_scan - 1)
            vector.wait_ge(pe_sem, pe_transpose)
            nc.vector.tensor_tensor_scan(
                prod[0:G, 0:K], pst[0:G, 0:K], zrow,
                initial=1.0,
                op0=mybir.AluOpType.mult, op1=mybir.AluOpType.bypass,
            ).then_inc(dve_sem, 1)

        @block.gpsimd
        def _(gpsimd):
            gpsimd.dma_start(cs[:, :], const_d[:, :]).then_inc(sem_const, 16)
            if fast_out:
                from concourse import library_config
                nc.gpsimd.load_library(library_config.mlp)
                # zero the scatter-add target (completion-sem ordered
                # before the trigger below); reads the cpack zeros region
                gpsimd.wait_ge(sem_const, 16)
                gpsimd.dma_start(out_d[:, :], zrow).then_inc(sem_zero, 16)
                gpsimd.wait_ge(sem_idx, 16)
                nc.gpsimd.dma_scatter_add(
                    out_ap=out_d[:, 0:1],
                    in_ap=prod[:, K - 1:K],
                    idxs_ap=idxs_sb[:, :],
                    num_idxs=128,
                    num_idxs_reg=G,
                    elem_size=1,
                    elem_step=64,
                    prepare_only=True,
                    sem=sem_out,
                ).then_inc(sem_prep, 1)
                gpsimd.wait_ge(sem_prep, 1)
                gpsimd.wait_ge(sem_zero, 16)
                gpsimd.wait_ge(dve_sem, dve_scan)
                gpsimd.trigger_dma(1)

    return nc


def _pack_consts(w1, b1, w2, b2, ow, ob):
    cp = np.zeros((128, CW), np.float32)
    cp[:, C_ID:C_ID + 128] = np.eye(128, dtype=np.float32)
    bv = cp.view(np.uint8)
    bf = ml_dtypes.bfloat16
    bv[:, C_W1B * 4:C_W1B * 4 + 256] = np.ascontiguousarray(
        w1.T.astype(bf)).view(np.uint8)
    bv[:, C_W2B * 4:C_W2B * 4 + 128] = np.ascontiguousarray(
        w2.T.astype(bf)).view(np.uint8)
    bv[DT:D1, C_OWB * 4:C_OWB * 4 + 2] = np.ascontiguousarray(
        ow.reshape(D2, 1).astype(bf)).view(np.uint8)
    cp[:, C_B1] = b1
    cp[DT:D1, C_B2] = b2
    cp[:, C_OB] = np.float32(ob.reshape(())[()])
    return cp


def _scatter_idxs():
    # scatter-add index table: position j (column-major over [16, ncols])
    # maps input partition j; the first G positions target output row g=j,
    # the rest are the ignored -1
    idx = np.full((128, 8), -1, np.int16)
    idx[0:G, 0] = np.arange(G, dtype=np.int16)
    return idx


def _make_in_maps(towers, x, w1, b1, w2, b2, ow, ob):
    towers = np.asarray(towers, np.float32)
    x = np.asarray(x, np.float32)
    cpack = _pack_consts(
        np.asarray(w1, np.float32), np.asarray(b1, np.float32),
        np.asarray(w2, np.float32), np.asarray(b2, np.float32),
        np.asarray(ow, np.float32), np.asarray(ob, np.float32),
    )
    xcT = np.concatenate(
        [towers.reshape(N * K, DT), x.reshape(N * K, D2)], axis=1
    ).T.astype(ml_dtypes.bfloat16)          # [128, N*K]
    in_maps = []
    for i in range(N_CORES):
        sl = slice(i * R, (i + 1) * R)
        m = {"cpack": cpack, "xc0T": np.ascontiguousarray(xcT[:, sl])}
        if FAST_OUT:
            m["idxs"] = _scatter_idxs()
        in_maps.append(m)
    return in_maps


def kernel(towers, x, w1, b1, w2, b2, aw1, ab1, aw2, ab2, ow, ob, k):
    global LAST_RESULT
    kk = int(k)

    if kk not in _PROGRAM_CACHE:
        nc = _build_program(kk)
        # encode pseudo ISA instructions (library reload) to raw ISA bytes;
        # walrus codegen rejects un-encoded pseudo instructions ("ISA wrong
        # length").  No effect on CoreSim timing or numerics.
        mybir.codegen_inst_isa_subclasses(nc)
        _PROGRAM_CACHE[kk] = nc
    nc = _PROGRAM_CACHE[kk]

    in_maps = _make_in_maps(towers, x, w1, b1, w2, b2, ow, ob)
    res = run_bass_kernel_spmd(nc, in_maps, list(range(N_CORES)))
    LAST_RESULT = res
    # out[g, 0] = product of graph g of that core
    out = np.concatenate([
        np.asarray(res.results[i]["out"])[:, 0].reshape(G)
        for i in range(N_CORES)
    ])
    return out.astype(np.float32)


# revision 79
# speedup vs baseline: 1.4564x; 1.0487x over previous
"""Trainium2 Bass kernel for nn_FCGAT (fully-connected GAT variant).

Mathematical simplifications (exact, same as the v1 kernel):

1. ``einsum('nkj,nkd->nkd', softmax(aa,2), h) == h`` (softmax rows sum to 1),
   so the whole attention block is dead code and the model reduces to::

       h_s = lrelu(lrelu([towers | x_s] @ w1.T + b1) @ w2.T + b2)
       x_{s+1} = h_s + x_s
       out_n = prod_k sigmoid(x_K @ ow[0] + ob[0])

2. Residuals are distributed through the linear maps as PSUM accumulation
   groups: ``x_s = x_0 + sum_{t<s} h_t`` so no elementwise adds exist.

v2 speedups over the 15.6us v1 kernel (CoreSim cost model):

* bf16 everywhere on the data path (inputs rounded on host, weights
  pre-rounded, hidden activations stored bf16; PSUM stays f32).  Halves the
  input DMA and makes every matmul 1 PE cycle/row.  Measured end-to-end
  numeric error vs a float64 oracle: 5.5e-3 (budget 2e-2).
* host-side transpose of the input to feature-major [D1, rows] removes the
  8 on-device PE transposes + DVE rounding copies of v1 entirely.
* 3 input DMAs ride 3 different queues (SP / ACT HWDGE + Pool SWDGE) which
  the cost model executes fully in parallel: all input sems fire at ~2.4us
  (the fixed DMA latency floor).
* Prelu (== leaky_relu with alpha) replaces Lrelu: parametric_relu lives in
  the SAME activation-table set as Sigmoid, so ONE table load (prewarmed at
  t=0 under the DMA shadow) covers the whole kernel.  v1 paid 2 prewarm
  loads plus a 1.3us mid-kernel reload for the final sigmoid.
* the logits matmul is restructured: instead of [1, R] output rows on one
  partition (v1: 612ns+ sigmoid, then a 1.5us DVE product tree), each
  128-row tile becomes a *stationary* operand against the moving ow column,
  yielding logits [128, 8] across partitions.  Sigmoid is then a 192ns ACT
  op; one PE transpose + two DVE cumprod scans (tensor_tensor_scan, op0=mult)
  produce the 16 per-graph products directly.

Sharding: data-parallel over N=128 -> 16 graphs (1024 rows) per core across
8 NeuronCores; weights replicated.

Raw Bass (explicit engine blocks + standalone single-condition waits), same
rationale as v1: the walrus build allows only one sync-wait per instruction.
"""

from contextlib import ExitStack

import ml_dtypes
import numpy as np

import concourse.bass as bass
import concourse.mybir as mybir
from concourse.bass_utils import run_bass_kernel_spmd

N_CORES = 8
N, K, DT, D2 = 128, 64, 64, 64
D1 = DT + D2                # 128: [towers | x] feature dim
G = N // N_CORES            # 16 graphs per core
R = G * K                   # 1024 rows per core
CHUNK = 512                 # psum-bank / moving-operand chunk
NCHUNK = R // CHUNK         # 2
TILE = 128                  # logits stationary tile (rows)
NT = R // TILE              # 8 logits tiles -> psum [128, NT]

# packed constants layout (f32 columns of a [128, CW] array; bf16 payloads
# are stored as raw bytes and bitcast on device)
C_ID = 0                    # 0:128    identity (f32, final transpose)
C_W1B = 128                 # 128:192  w1.T as bf16 [128, 128]
C_W2B = 192                 # 192:224  w2.T as bf16 [128, 64]
C_OWB = 224                 # col 224, rows 64:128: ow[0] as bf16 (low half)
C_B1 = 225                  # b1 f32 column
C_B2 = 226                  # b2 f32 column, rows 64:128
C_OB = 227                  # ob f32 replicated on all 128 rows
C_Z = 228                   # 228:292  zeros (f32; scan dummy + output zeroing)
CW = 292

_F32 = mybir.dt.float32
_BF16 = mybir.dt.bfloat16

# FAST_OUT (SWDGE scatter-add + trigger output path) cuts ~2.1us off the
# CoreSim tail but the gpsimd dma_scatter_add Q7 kernel crashes the device
# on this terminal's firmware (verified with isolated probes in both
# prepare/trigger and immediate modes; a plain load_library works).  Keep
# the implementation but ship with the plain HWDGE output DMA.
FAST_OUT = False
DVE_COLS = 192
DVE_H1_COLS = 0

LAST_RESULT = None
_PROGRAM_CACHE = {}


def _build_program(kk: int, act_fn=None, sig_fn=None, out_sync=2,
                   fast_out=FAST_OUT, dve_cols=DVE_COLS,
                   dve_h1_cols=None) -> bass.Bass:
    """act_fn/sig_fn overrides exist for CoreSim exec-mode validation
    (CoreSim implements Relu/Sigmoid but not Prelu).

    out_sync: 2 = output DMA incs sem_out and SP waits on it (fully safe);
    1 = inc only, no wait; 0 = no completion sem at all.

    dve_cols: offload the last dve_cols columns of every h2 chunk from the
    saturated ACT engine to the otherwise-idle DVE as
    max(x + b2, 0.01*(x + b2)) (two DVE ops).  Must be a multiple of 64 so
    the per-graph logits slices don't straddle the ACT/DVE boundary.

    fast_out: write the 16 products with a SWDGE scatter-add whose
    descriptors are prepared at t~1us and fired by trigger_dma at the end.
    The trigger path skips the 625ns HWDGE + 650ns DGE delay of a regular
    DMA, cutting ~1.2us off the tail.  Scatter-add ACCUMULATES into HBM, so
    the output buffer is zeroed by an early DMA (completion-sem ordered
    before the trigger).  fast_out=False falls back to a plain SP-queue
    DMA."""
    PRELU = act_fn or mybir.ActivationFunctionType.Prelu
    SIGMOID = sig_fn or mybir.ActivationFunctionType.Sigmoid
    if dve_h1_cols is None:
        dve_h1_cols = DVE_H1_COLS
    DD = dve_cols                   # DVE columns per h2 chunk
    AA = CHUNK - DD                 # ACT columns per h2 chunk
    HH = dve_h1_cols                # DVE columns per h1 chunk
    AH = CHUNK - HH                 # ACT columns per h1 chunk
    assert DD % 64 == 0 and 0 <= DD < CHUNK
    assert HH % 64 == 0 and 0 <= HH < CHUNK

    nc = bass.Bass()
    const_d = nc.declare_dram_parameter("cpack", [128, CW], _F32,
                                        isOutput=False)
    xd = nc.declare_dram_parameter("xc0T", [128, R], _BF16, isOutput=False)
    out_d = nc.declare_dram_parameter("out", [G, 64 if fast_out else 1],
                                      _F32, isOutput=True)
    idx_d = (nc.declare_dram_parameter("idxs", [128, 8], mybir.dt.int16,
                                       isOutput=False) if fast_out else None)

    with ExitStack() as ctx:
        cs = ctx.enter_context(nc.sbuf_tensor([128, CW], _F32))
        idxs_sb = ctx.enter_context(
            nc.sbuf_tensor("idxs_sb", [128, 8], mybir.dt.int16))
        tmp = ctx.enter_context(
            nc.sbuf_tensor("tmp", [128, max(1, 2 * kk * max(DD, 1))], _BF16))
        tmph = ctx.enter_context(
            nc.sbuf_tensor("tmph", [128, max(1, 2 * kk * max(HH, 1))],
                           _BF16))
        xcT = ctx.enter_context(nc.sbuf_tensor([128, R], _BF16))
        h1s = ctx.enter_context(nc.sbuf_tensor([128, R], _BF16))
        h2s = [ctx.enter_context(nc.sbuf_tensor(f"h2s_{s}", [128, R], _BF16))
               for s in range(kk)]
        sig = ctx.enter_context(nc.sbuf_tensor([128, G], _F32))
        prod = ctx.enter_context(nc.sbuf_tensor([128, K], _F32))
        warm = ctx.enter_context(nc.sbuf_tensor([1, 1], _F32))
        # full-bank psum allocations (avoid same-bank PE-write/engine-read)
        ps1 = [ctx.enter_context(nc.psum_tensor(f"ps1_{c}", [128, 512], _F32))
               for c in range(NCHUNK)]
        ps2 = [ctx.enter_context(nc.psum_tensor(f"ps2_{c}", [128, 512], _F32))
               for c in range(NCHUNK)]
        ps3 = ctx.enter_context(nc.psum_tensor([128, 512], _F32))
        pst = ctx.enter_context(nc.psum_tensor([128, 512], _F32))
        psb = [ps3, pst]        # DVE-read h2 slices borrow these banks
        sem_const = ctx.enter_context(nc.semaphore("sem_const"))
        sem_d = [ctx.enter_context(nc.semaphore(f"sem_d{c}"))
                 for c in range(NCHUNK)]
        sem_out = ctx.enter_context(nc.semaphore("sem_out"))
        sem_zero = ctx.enter_context(nc.semaphore("sem_zero"))
        sem_prep = ctx.enter_context(nc.semaphore("sem_prep"))
        sem_idx = ctx.enter_context(nc.semaphore("sem_idx"))
        pe_sem = ctx.enter_context(nc.semaphore("pe_sem"))
        act_sem = ctx.enter_context(nc.semaphore("act_sem"))
        dve_sem = ctx.enter_context(nc.semaphore("dve_sem"))
        block = ctx.enter_context(nc.Block())

        ident = cs[:, C_ID:C_ID + 128]
        w1t = cs[:, C_W1B:C_W1B + 64].bitcast(_BF16)        # [128, 128]
        w2t = cs[:, C_W2B:C_W2B + 32].bitcast(_BF16)        # [128, 64]
        owc = cs[DT:D1, C_OWB:C_OWB + 1].bitcast(_BF16)[:, 0:1]  # [64, 1]
        b1 = cs[:, C_B1:C_B1 + 1]
        b2 = cs[DT:D1, C_B2:C_B2 + 1]
        obc = cs[:, C_OB:C_OB + 1]
        zrow = cs[0:G, C_Z:C_Z + K]                         # [16, 64] zeros

        def csl(c):
            return slice(c * CHUNK, (c + 1) * CHUNK)

        # ---- PE instruction numbering (emitted below in this order) ----
        pe_n = 0
        pe_mm1 = {}   # (s, c) -> pe value after the mm1 group for (s, c)
        pe_mm2 = {}   # (s, c)
        pe_mm2b = {}  # (s, c) DVE-bank slice
        pe_mm3_last = 0
        pe_transpose = 0
        act_n = 1     # warm == 1
        act_h1 = {}
        act_h2 = {}
        for s in range(kk):
            for c in range(NCHUNK):
                act_h1[(s, c)] = act_n + 1
                act_n += 1
            for c in range(NCHUNK):
                act_h2[(s, c)] = act_n + 1
                act_n += 1
        act_sig = act_n + 1
        # DVE numbering: memset (fast_out), then per step the h1-offload
        # pairs then the h2-offload pairs, then the cumprod scan
        dve_n = 1 if fast_out else 0
        dve_h1b = {}
        dve_h2b = {}
        for s in range(kk):
            for c in range(NCHUNK):
                if HH:
                    dve_n += 2
                    dve_h1b[(s, c)] = dve_n
            for c in range(NCHUNK):
                if DD:
                    dve_n += 2
                    dve_h2b[(s, c)] = dve_n
        dve_scan = dve_n + 1

        @block.sync
        def _(sync):
            sync.dma_start(xcT[:, csl(0)], xd[:, csl(0)]).then_inc(
                sem_d[0], 16)
            if fast_out:
                sync.dma_start(idxs_sb[:, :], idx_d[:, :]).then_inc(
                    sem_idx, 16)
            if not fast_out:
                sync.wait_ge(dve_sem, dve_scan)
                dma = sync.dma_start(out_d[:, :], prod[0:G, K - 1:K])
                if out_sync >= 1:
                    dma.then_inc(sem_out, 16)
            if out_sync >= 2:
                sync.wait_ge(sem_out, 16)

        @block.tensor
        def _(tensor):
            nonlocal pe_mm3_last, pe_transpose
            wm = {}

            def twait(sem, val):
                if wm.get(id(sem), 0) < val:
                    wm[id(sem)] = val
                    tensor.wait_ge(sem, val)

            pe = 0

            def inc(instr):
                nonlocal pe
                pe += 1
                instr.then_inc(pe_sem, 1)

            def mm3_column(g):
                # one complete accumulation group per psum column (one GRAPH
                # per column: 64 rows -> 64 output partitions): the PSUM
                # start bit zeroes the whole 2KB bank region, so groups in a
                # bank must be strictly sequential, not interleaved
                gsl = slice(g * K, (g + 1) * K)
                terms = [xcT[DT:D1, gsl]] + [
                    h2s[s][DT:D1, gsl] for s in range(kk)]
                for i, lhsT in enumerate(terms):
                    inc(nc.tensor.matmul(
                        ps3[0:K, g:g + 1], lhsT, owc,
                        start=(i == 0), stop=(i == len(terms) - 1),
                    ))

            # step-0 mm1
            twait(sem_const, 16)
            for c in range(NCHUNK):
                twait(sem_d[c], 16)
                inc(nc.tensor.matmul(
                    ps1[c][:, :], w1t, xcT[:, csl(c)],
                    start=True, stop=True))
                pe_mm1[(0, c)] = pe
            for s in range(kk):
                last_step = (s == kk - 1)
                for c in range(NCHUNK):
                    lo = c * CHUNK
                    twait(act_sem, act_h1[(s, c)])
                    if HH:
                        twait(dve_sem, dve_h1b[(s, c)])
                    if s >= 1 and DD:
                        # DVE bank WAR vs previous step's DVE h2 reader
                        twait(dve_sem, dve_h2b[(s - 1, c)])
                    if DD:
                        # concurrent ACT+DVE reads of one PSUM bank hang
                        # the device (verified by an isolated probe), so
                        # mm2 splits: the ACT slice lands in ps2[c], the
                        # DVE slice in the late-kernel ps3/pst banks
                        inc(nc.tensor.matmul(
                            ps2[c][DT:D1, 0:AA], w2t, h1s[:, lo:lo + AA],
                            start=True, stop=True))
                        pe_mm2[(s, c)] = pe
                        inc(nc.tensor.matmul(
                            psb[c][DT:D1, CHUNK - DD:CHUNK], w2t,
                            h1s[:, lo + AA:lo + CHUNK],
                            start=True, stop=True))
                        pe_mm2b[(s, c)] = pe
                    else:
                        inc(nc.tensor.matmul(
                            ps2[c][DT:D1, :], w2t, h1s[:, csl(c)],
                            start=True, stop=True))
                        pe_mm2[(s, c)] = pe
                    if not last_step:
                        # next-step mm1 x0 part; ps1[c] WAR cleared by the
                        # act_h1 wait just above
                        inc(nc.tensor.matmul(
                            ps1[c][:, :], w1t, xcT[:, csl(c)],
                            start=True, stop=False))
                if not last_step:
                    for c in range(NCHUNK):
                        lo = c * CHUNK
                        for t in range(s):
                            inc(nc.tensor.matmul(
                                ps1[c][:, :], w1t[DT:D1, :],
                                h2s[t][DT:D1, csl(c)],
                                start=False, stop=False))
                        # term t == s arrives in two pieces when the h2
                        # activation is split across ACT and DVE; wait for
                        # both producers, then one full-width matmul
                        twait(act_sem, act_h2[(s, c)])
                        if DD:
                            twait(dve_sem, dve_h2b[(s, c)])
                        inc(nc.tensor.matmul(
                            ps1[c][:, :], w1t[DT:D1, :],
                            h2s[s][DT:D1, csl(c)],
                            start=False, stop=True))
                        pe_mm1[(s + 1, c)] = pe
            for c in range(NCHUNK):
                if kk > 0:
                    twait(act_sem, act_h2[(kk - 1, c)])
                    if DD:
                        twait(dve_sem, dve_h2b[(kk - 1, c)])
                for g in range(c * (G // NCHUNK), (c + 1) * (G // NCHUNK)):
                    mm3_column(g)
            pe_mm3_last = pe
            twait(act_sem, act_sig)
            inc(nc.tensor.transpose(
                pst[0:G, 0:K], sig[0:K, 0:G], ident[0:K, 0:K]))
            pe_transpose = pe

        @block.scalar
        def _(scalar):
            scalar.dma_start(xcT[:, csl(1)], xd[:, csl(1)]).then_inc(
                sem_d[1], 16)
            zcell = nc.const_aps.aps[(mybir.dt.float32, 0.0)][0:1, 0:1]
            nc.scalar.activation(warm[0:1, 0:1], zcell, SIGMOID).then_inc(
                act_sem, 1)
            seen = 0

            def swait(val):
                nonlocal seen
                if val > seen:
                    seen = val
                    scalar.wait_ge(pe_sem, val)

            for s in range(kk):
                for c in range(NCHUNK):
                    lo = c * CHUNK
                    swait(pe_mm1[(s, c)])
                    nc.scalar.activation(
                        h1s[:, lo:lo + AH], ps1[c][:, 0:AH], PRELU,
                        bias=b1, alpha=0.01,
                    ).then_inc(act_sem, 1)
                for c in range(NCHUNK):
                    lo = c * CHUNK
                    swait(pe_mm2[(s, c)])
                    nc.scalar.activation(
                        h2s[s][DT:D1, lo:lo + AA], ps2[c][DT:D1, 0:AA],
                        PRELU, bias=b2, alpha=0.01,
                    ).then_inc(act_sem, 1)
            swait(pe_mm3_last)
            nc.scalar.activation(
                sig[0:K, 0:G], ps3[0:K, 0:G], SIGMOID, bias=obc[0:K, :],
            ).then_inc(act_sem, 1)

        @block.vector
        def _(vector):
            if fast_out:
                # the scatter-add src AP spans all 128 partitions; only
                # 0:G hold real products, the rest must be initialized
                # (they map to ignored negative indices).  Engines need
                # quadrant-aligned base partitions, so clear the full
                # column up front and let the scan overwrite rows 0:G.
                nc.vector.memset(prod[:, K - 1:K], 0.0).then_inc(
                    dve_sem, 1)
            # lrelu offload pairs: out = max(x + b, 0.01*(x + b))
            for s in range(kk):
                for c in range(NCHUNK):
                    if not HH:
                        continue
                    lo = c * CHUNK
                    tsl = slice((s * NCHUNK + c) * HH,
                                (s * NCHUNK + c + 1) * HH)
                    vector.wait_ge(pe_sem, pe_mm1[(s, c)])
                    nc.vector.tensor_scalar(
                        out=tmph[:, tsl], in0=ps1[c][:, AH:CHUNK],
                        scalar1=b1, scalar2=0.01,
                        op0=mybir.AluOpType.add, op1=mybir.AluOpType.mult,
                    ).then_inc(dve_sem, 1)
                    vector.wait_ge(dve_sem, dve_h1b[(s, c)] - 1)
                    nc.vector.scalar_tensor_tensor(
                        out=h1s[:, lo + AH:lo + CHUNK],
                        in0=ps1[c][:, AH:CHUNK], scalar=b1,
                        in1=tmph[:, tsl],
                        op0=mybir.AluOpType.add, op1=mybir.AluOpType.max,
                    ).then_inc(dve_sem, 1)
                for c in range(NCHUNK):
                    if not DD:
                        continue
                    lo = c * CHUNK
                    tsl = slice((s * NCHUNK + c) * DD,
                                (s * NCHUNK + c + 1) * DD)
                    vector.wait_ge(pe_sem, pe_mm2b[(s, c)])
                    nc.vector.tensor_scalar(
                        out=tmp[DT:D1, tsl],
                        in0=psb[c][DT:D1, CHUNK - DD:CHUNK],
                        scalar1=b2, scalar2=0.01,
                        op0=mybir.AluOpType.add, op1=mybir.AluOpType.mult,
                    ).then_inc(dve_sem, 1)
                    # explicit completion wait: DVE RAW on tmp
                    vector.wait_ge(dve_sem, dve_h2b[(s, c)] - 1)
                    nc.vector.scalar_tensor_tensor(
                        out=h2s[s][DT:D1, lo + AA:lo + CHUNK],
                        in0=psb[c][DT:D1, CHUNK - DD:CHUNK], scalar=b2,
                        in1=tmp[DT:D1, tsl],
                        op0=mybir.AluOpType.add, op1=mybir.AluOpType.max,
                    ).then_inc(dve_sem, 1)
            # explicit ordering: the scan below rewrites rows 0:G of the
            # memset column
            vector.wait_ge(dve_sem, dve_scan - 1)
            vector.wait_ge(pe_sem, pe_transpose)
            nc.vector.tensor_tensor_scan(
                prod[0:G, 0:K], pst[0:G, 0:K], zrow,
                initial=1.0,
                op0=mybir.AluOpType.mult, op1=mybir.AluOpType.bypass,
            ).then_inc(dve_sem, 1)

        @block.gpsimd
        def _(gpsimd):
            gpsimd.dma_start(cs[:, :], const_d[:, :]).then_inc(sem_const, 16)
            if fast_out:
                from concourse import library_config
                nc.gpsimd.load_library(library_config.mlp)
                # zero the scatter-add target (completion-sem ordered
                # before the trigger below); reads the cpack zeros region
                gpsimd.wait_ge(sem_const, 16)
                gpsimd.dma_start(out_d[:, :], zrow).then_inc(sem_zero, 16)
                gpsimd.wait_ge(sem_idx, 16)
                nc.gpsimd.dma_scatter_add(
                    out_ap=out_d[:, 0:1],
                    in_ap=prod[:, K - 1:K],
                    idxs_ap=idxs_sb[:, :],
                    num_idxs=128,
                    num_idxs_reg=G,
                    elem_size=1,
                    elem_step=64,
                    prepare_only=True,
                    sem=sem_out,
                ).then_inc(sem_prep, 1)
                gpsimd.wait_ge(sem_prep, 1)
                gpsimd.wait_ge(sem_zero, 16)
                gpsimd.wait_ge(dve_sem, dve_scan)
                gpsimd.trigger_dma(1)

    return nc


def _pack_consts(w1, b1, w2, b2, ow, ob):
    cp = np.zeros((128, CW), np.float32)
    cp[:, C_ID:C_ID + 128] = np.eye(128, dtype=np.float32)
    bv = cp.view(np.uint8)
    bf = ml_dtypes.bfloat16
    bv[:, C_W1B * 4:C_W1B * 4 + 256] = np.ascontiguousarray(
        w1.T.astype(bf)).view(np.uint8)
    bv[:, C_W2B * 4:C_W2B * 4 + 128] = np.ascontiguousarray(
        w2.T.astype(bf)).view(np.uint8)
    bv[DT:D1, C_OWB * 4:C_OWB * 4 + 2] = np.ascontiguousarray(
        ow.reshape(D2, 1).astype(bf)).view(np.uint8)
    cp[:, C_B1] = b1
    cp[DT:D1, C_B2] = b2
    cp[:, C_OB] = np.float32(ob.reshape(())[()])
    return cp


def _scatter_idxs():
    # scatter-add index table: position j (column-major over [16, ncols])
    # maps input partition j; the first G positions target output row g=j,
    # the rest are the ignored -1
    idx = np.full((128, 8), -1, np.int16)
    idx[0:G, 0] = np.arange(G, dtype=np.int16)
    return idx


def _make_in_maps(towers, x, w1, b1, w2, b2, ow, ob):
    towers = np.asarray(towers, np.float32)
    x = np.asarray(x, np.float32)
    cpack = _pack_consts(
        np.asarray(w1, np.float32), np.asarray(b1, np.float32),
        np.asarray(w2, np.float32), np.asarray(b2, np.float32),
        np.asarray(ow, np.float32), np.asarray(ob, np.float32),
    )
    xcT = np.concatenate(
        [towers.reshape(N * K, DT), x.reshape(N * K, D2)], axis=1
    ).T.astype(ml_dtypes.bfloat16)          # [128, N*K]
    in_maps = []
    for i in range(N_CORES):
        sl = slice(i * R, (i + 1) * R)
        m = {"cpack": cpack, "xc0T": np.ascontiguousarray(xcT[:, sl])}
        if FAST_OUT:
            m["idxs"] = _scatter_idxs()
        in_maps.append(m)
    return in_maps


def kernel(towers, x, w1, b1, w2, b2, aw1, ab1, aw2, ab2, ow, ob, k):
    global LAST_RESULT
    kk = int(k)

    if kk not in _PROGRAM_CACHE:
        nc = _build_program(kk)
        # encode pseudo ISA instructions (library reload) to raw ISA bytes;
        # walrus codegen rejects un-encoded pseudo instructions ("ISA wrong
        # length").  No effect on CoreSim timing or numerics.
        mybir.codegen_inst_isa_subclasses(nc)
        _PROGRAM_CACHE[kk] = nc
    nc = _PROGRAM_CACHE[kk]

    in_maps = _make_in_maps(towers, x, w1, b1, w2, b2, ow, ob)
    res = run_bass_kernel_spmd(nc, in_maps, list(range(N_CORES)))
    LAST_RESULT = res
    # out[g, 0] = product of graph g of that core
    out = np.concatenate([
        np.asarray(res.results[i]["out"])[:, 0].reshape(G)
        for i in range(N_CORES)
    ])
    return out.astype(np.float32)


# revision 87
# speedup vs baseline: 1.4904x; 1.0233x over previous
"""Trainium2 Bass kernel for nn_FCGAT (fully-connected GAT variant).

Mathematical simplifications (exact, same as the v1 kernel):

1. ``einsum('nkj,nkd->nkd', softmax(aa,2), h) == h`` (softmax rows sum to 1),
   so the whole attention block is dead code and the model reduces to::

       h_s = lrelu(lrelu([towers | x_s] @ w1.T + b1) @ w2.T + b2)
       x_{s+1} = h_s + x_s
       out_n = prod_k sigmoid(x_K @ ow[0] + ob[0])

2. Residuals are distributed through the linear maps as PSUM accumulation
   groups: ``x_s = x_0 + sum_{t<s} h_t`` so no elementwise adds exist.

v2 speedups over the 15.6us v1 kernel (CoreSim cost model):

* bf16 everywhere on the data path (inputs rounded on host, weights
  pre-rounded, hidden activations stored bf16; PSUM stays f32).  Halves the
  input DMA and makes every matmul 1 PE cycle/row.  Measured end-to-end
  numeric error vs a float64 oracle: 5.5e-3 (budget 2e-2).
* host-side transpose of the input to feature-major [D1, rows] removes the
  8 on-device PE transposes + DVE rounding copies of v1 entirely.
* 3 input DMAs ride 3 different queues (SP / ACT HWDGE + Pool SWDGE) which
  the cost model executes fully in parallel: all input sems fire at ~2.4us
  (the fixed DMA latency floor).
* Prelu (== leaky_relu with alpha) replaces Lrelu: parametric_relu lives in
  the SAME activation-table set as Sigmoid, so ONE table load (prewarmed at
  t=0 under the DMA shadow) covers the whole kernel.  v1 paid 2 prewarm
  loads plus a 1.3us mid-kernel reload for the final sigmoid.
* the logits matmul is restructured: instead of [1, R] output rows on one
  partition (v1: 612ns+ sigmoid, then a 1.5us DVE product tree), each
  128-row tile becomes a *stationary* operand against the moving ow column,
  yielding logits [128, 8] across partitions.  Sigmoid is then a 192ns ACT
  op; one PE transpose + two DVE cumprod scans (tensor_tensor_scan, op0=mult)
  produce the 16 per-graph products directly.

Sharding: data-parallel over N=128 -> 16 graphs (1024 rows) per core across
8 NeuronCores; weights replicated.

Raw Bass (explicit engine blocks + standalone single-condition waits), same
rationale as v1: the walrus build allows only one sync-wait per instruction.
"""

from contextlib import ExitStack

import ml_dtypes
import numpy as np

import concourse.bass as bass
import concourse.mybir as mybir
from concourse.bass_utils import run_bass_kernel_spmd

N_CORES = 8
N, K, DT, D2 = 128, 64, 64, 64
D1 = DT + D2                # 128: [towers | x] feature dim
G = N // N_CORES            # 16 graphs per core
R = G * K                   # 1024 rows per core
CHUNK = 512                 # psum-bank / moving-operand chunk
NCHUNK = R // CHUNK         # 2
TILE = 128                  # logits stationary tile (rows)
NT = R // TILE              # 8 logits tiles -> psum [128, NT]

# packed constants layout (f32 columns of a [128, CW] array; bf16 payloads
# are stored as raw bytes and bitcast on device)
C_ID = 0                    # 0:128    identity (f32, final transpose)
C_W1B = 128                 # 128:192  w1.T as bf16 [128, 128]
C_W2B = 192                 # 192:224  w2.T as bf16 [128, 64]
C_OWB = 224                 # col 224, rows 64:128: ow[0] as bf16 (low half)
C_B1 = 225                  # b1 f32 column
C_B2 = 226                  # b2 f32 column, rows 64:128
C_OB = 227                  # ob f32 replicated on all 128 rows
C_Z = 228                   # 228:292  zeros (f32; scan dummy + output zeroing)
CW = 292

_F32 = mybir.dt.float32
_BF16 = mybir.dt.bfloat16

# FAST_OUT (SWDGE scatter-add + trigger output path) cuts ~2.1us off the
# CoreSim tail but the gpsimd dma_scatter_add Q7 kernel crashes the device
# on this terminal's firmware (verified with isolated probes in both
# prepare/trigger and immediate modes; a plain load_library works).  Keep
# the implementation but ship with the plain HWDGE output DMA.
FAST_OUT = False
DVE_COLS = 128
DVE_H1_COLS = 128

LAST_RESULT = None
_PROGRAM_CACHE = {}


def _build_program(kk: int, act_fn=None, sig_fn=None, out_sync=2,
                   fast_out=FAST_OUT, dve_cols=DVE_COLS,
                   dve_h1_cols=None) -> bass.Bass:
    """act_fn/sig_fn overrides exist for CoreSim exec-mode validation
    (CoreSim implements Relu/Sigmoid but not Prelu).

    out_sync: 2 = output DMA incs sem_out and SP waits on it (fully safe);
    1 = inc only, no wait; 0 = no completion sem at all.

    dve_cols: offload the last dve_cols columns of every h2 chunk from the
    saturated ACT engine to the otherwise-idle DVE as
    max(x + b2, 0.01*(x + b2)) (two DVE ops).  Must be a multiple of 64 so
    the per-graph logits slices don't straddle the ACT/DVE boundary.

    fast_out: write the 16 products with a SWDGE scatter-add whose
    descriptors are prepared at t~1us and fired by trigger_dma at the end.
    The trigger path skips the 625ns HWDGE + 650ns DGE delay of a regular
    DMA, cutting ~1.2us off the tail.  Scatter-add ACCUMULATES into HBM, so
    the output buffer is zeroed by an early DMA (completion-sem ordered
    before the trigger).  fast_out=False falls back to a plain SP-queue
    DMA."""
    PRELU = act_fn or mybir.ActivationFunctionType.Prelu
    SIGMOID = sig_fn or mybir.ActivationFunctionType.Sigmoid
    if dve_h1_cols is None:
        dve_h1_cols = DVE_H1_COLS
    DD = dve_cols                   # DVE columns per h2 chunk
    AA = CHUNK - DD                 # ACT columns per h2 chunk
    HH = dve_h1_cols                # DVE columns per h1 chunk
    AH = CHUNK - HH                 # ACT columns per h1 chunk
    assert DD % 64 == 0 and 0 <= DD < CHUNK
    assert HH % 64 == 0 and 0 <= HH < CHUNK

    nc = bass.Bass()
    const_d = nc.declare_dram_parameter("cpack", [128, CW], _F32,
                                        isOutput=False)
    xd = nc.declare_dram_parameter("xc0T", [128, R], _BF16, isOutput=False)
    out_d = nc.declare_dram_parameter("out", [G, 64 if fast_out else 1],
                                      _F32, isOutput=True)
    idx_d = (nc.declare_dram_parameter("idxs", [128, 8], mybir.dt.int16,
                                       isOutput=False) if fast_out else None)

    with ExitStack() as ctx:
        cs = ctx.enter_context(nc.sbuf_tensor([128, CW], _F32))
        idxs_sb = ctx.enter_context(
            nc.sbuf_tensor("idxs_sb", [128, 8], mybir.dt.int16))
        tmp = ctx.enter_context(
            nc.sbuf_tensor("tmp", [128, max(1, 2 * kk * max(DD, 1))], _BF16))
        tmph = ctx.enter_context(
            nc.sbuf_tensor("tmph", [128, max(1, 2 * kk * max(HH, 1))],
                           _BF16))
        xcT = ctx.enter_context(nc.sbuf_tensor([128, R], _BF16))
        h1s = ctx.enter_context(nc.sbuf_tensor([128, R], _BF16))
        h2s = [ctx.enter_context(nc.sbuf_tensor(f"h2s_{s}", [128, R], _BF16))
               for s in range(kk)]
        sig = ctx.enter_context(nc.sbuf_tensor([128, G], _F32))
        prod = ctx.enter_context(nc.sbuf_tensor([128, K], _F32))
        warm = ctx.enter_context(nc.sbuf_tensor([1, 1], _F32))
        # full-bank psum allocations (avoid same-bank PE-write/engine-read)
        ps1 = [ctx.enter_context(nc.psum_tensor(f"ps1_{c}", [128, 512], _F32))
               for c in range(NCHUNK)]
        ps2 = [ctx.enter_context(nc.psum_tensor(f"ps2_{c}", [128, 512], _F32))
               for c in range(NCHUNK)]
        ps3 = ctx.enter_context(nc.psum_tensor([128, 512], _F32))
        pst = ctx.enter_context(nc.psum_tensor([128, 512], _F32))
        psb = [ps3, pst]        # DVE-read h2 slices borrow these banks
        ps1b = [ctx.enter_context(
            nc.psum_tensor(f"ps1b_{c}", [128, 512], _F32))
            for c in range(NCHUNK if HH else 0)]
        sem_const = ctx.enter_context(nc.semaphore("sem_const"))
        sem_d = [ctx.enter_context(nc.semaphore(f"sem_d{c}"))
                 for c in range(NCHUNK)]
        sem_out = ctx.enter_context(nc.semaphore("sem_out"))
        sem_zero = ctx.enter_context(nc.semaphore("sem_zero"))
        sem_prep = ctx.enter_context(nc.semaphore("sem_prep"))
        sem_idx = ctx.enter_context(nc.semaphore("sem_idx"))
        pe_sem = ctx.enter_context(nc.semaphore("pe_sem"))
        act_sem = ctx.enter_context(nc.semaphore("act_sem"))
        dve_sem = ctx.enter_context(nc.semaphore("dve_sem"))
        block = ctx.enter_context(nc.Block())

        ident = cs[:, C_ID:C_ID + 128]
        w1t = cs[:, C_W1B:C_W1B + 64].bitcast(_BF16)        # [128, 128]
        w2t = cs[:, C_W2B:C_W2B + 32].bitcast(_BF16)        # [128, 64]
        owc = cs[DT:D1, C_OWB:C_OWB + 1].bitcast(_BF16)[:, 0:1]  # [64, 1]
        b1 = cs[:, C_B1:C_B1 + 1]
        b2 = cs[DT:D1, C_B2:C_B2 + 1]
        obc = cs[:, C_OB:C_OB + 1]
        zrow = cs[0:G, C_Z:C_Z + K]                         # [16, 64] zeros

        def csl(c):
            return slice(c * CHUNK, (c + 1) * CHUNK)

        # ---- PE instruction numbering (emitted below in this order) ----
        pe_n = 0
        pe_mm1 = {}   # (s, c) -> pe value after the mm1 group for (s, c)
        pe_mm1b = {}  # (s, c) DVE-bank h1 slice
        pe_mm2 = {}   # (s, c)
        pe_mm2b = {}  # (s, c) DVE-bank slice
        pe_mm3_last = 0
        pe_transpose = 0
        act_n = 1     # warm == 1
        act_h1 = {}
        act_h2 = {}
        for s in range(kk):
            for c in range(NCHUNK):
                act_h1[(s, c)] = act_n + 1
                act_n += 1
            for c in range(NCHUNK):
                act_h2[(s, c)] = act_n + 1
                act_n += 1
        act_sig = act_n + 1
        # DVE numbering: memset (fast_out), then per step the h1-offload
        # pairs then the h2-offload pairs, then the cumprod scan
        dve_n = 1 if fast_out else 0
        dve_h1b = {}
        dve_h2b = {}
        for s in range(kk):
            for c in range(NCHUNK):
                if HH:
                    dve_n += 2
                    dve_h1b[(s, c)] = dve_n
            for c in range(NCHUNK):
                if DD:
                    dve_n += 2
                    dve_h2b[(s, c)] = dve_n
        dve_scan = dve_n + 1

        @block.sync
        def _(sync):
            sync.dma_start(xcT[:, csl(0)], xd[:, csl(0)]).then_inc(
                sem_d[0], 16)
            if fast_out:
                sync.dma_start(idxs_sb[:, :], idx_d[:, :]).then_inc(
                    sem_idx, 16)
            if not fast_out:
                sync.wait_ge(dve_sem, dve_scan)
                dma = sync.dma_start(out_d[:, :], prod[0:G, K - 1:K])
                if out_sync >= 1:
                    dma.then_inc(sem_out, 16)
            if out_sync >= 2:
                sync.wait_ge(sem_out, 16)

        @block.tensor
        def _(tensor):
            nonlocal pe_mm3_last, pe_transpose
            wm = {}

            def twait(sem, val):
                if wm.get(id(sem), 0) < val:
                    wm[id(sem)] = val
                    tensor.wait_ge(sem, val)

            pe = 0

            def inc(instr):
                nonlocal pe
                pe += 1
                instr.then_inc(pe_sem, 1)

            def mm3_column(g):
                # one complete accumulation group per psum column (one GRAPH
                # per column: 64 rows -> 64 output partitions): the PSUM
                # start bit zeroes the whole 2KB bank region, so groups in a
                # bank must be strictly sequential, not interleaved
                gsl = slice(g * K, (g + 1) * K)
                terms = [xcT[DT:D1, gsl]] + [
                    h2s[s][DT:D1, gsl] for s in range(kk)]
                for i, lhsT in enumerate(terms):
                    inc(nc.tensor.matmul(
                        ps3[0:K, g:g + 1], lhsT, owc,
                        start=(i == 0), stop=(i == len(terms) - 1),
                    ))

            def mm1_group(s, c, is_first):
                # moving columns split: group-a (ACT's bank), group-b
                # (DVE's bank) -- same PSUM-contention fix as mm2
                lo = c * CHUNK
                parts = [(ps1[c][:, 0:AH], slice(lo, lo + AH))]
                if HH:
                    parts.append(
                        (ps1b[c][:, 0:HH], slice(lo + AH, lo + CHUNK)))
                pes = []
                for dst, msl in parts:
                    inc(nc.tensor.matmul(
                        dst, w1t, xcT[:, msl],
                        start=True, stop=is_first))
                    pes.append(pe)
                return pes

            def mm1_h2part(s, c, t, last):
                lo = c * CHUNK
                parts = [(ps1[c][:, 0:AH], slice(lo, lo + AH))]
                if HH:
                    parts.append(
                        (ps1b[c][:, 0:HH], slice(lo + AH, lo + CHUNK)))
                pes = []
                for dst, msl in parts:
                    inc(nc.tensor.matmul(
                        dst, w1t[DT:D1, :], h2s[t][DT:D1, msl],
                        start=False, stop=last))
                    pes.append(pe)
                return pes

            # step-0 mm1
            twait(sem_const, 16)
            for c in range(NCHUNK):
                twait(sem_d[c], 16)
                pes = mm1_group(0, c, is_first=True)
                pe_mm1[(0, c)] = pes[0]
                pe_mm1b[(0, c)] = pes[-1]
            for s in range(kk):
                last_step = (s == kk - 1)
                for c in range(NCHUNK):
                    lo = c * CHUNK
                    twait(act_sem, act_h1[(s, c)])
                    if HH:
                        twait(dve_sem, dve_h1b[(s, c)])
                    if s >= 1 and DD:
                        # DVE bank WAR vs previous step's DVE h2 reader
                        twait(dve_sem, dve_h2b[(s - 1, c)])
                    if DD:
                        # concurrent ACT+DVE reads of one PSUM bank hang
                        # the device (verified by an isolated probe), so
                        # mm2 splits: the ACT slice lands in ps2[c], the
                        # DVE slice in the late-kernel ps3/pst banks
                        inc(nc.tensor.matmul(
                            ps2[c][DT:D1, 0:AA], w2t, h1s[:, lo:lo + AA],
                            start=True, stop=True))
                        pe_mm2[(s, c)] = pe
                        inc(nc.tensor.matmul(
                            psb[c][DT:D1, CHUNK - DD:CHUNK], w2t,
                            h1s[:, lo + AA:lo + CHUNK],
                            start=True, stop=True))
                        pe_mm2b[(s, c)] = pe
                    else:
                        inc(nc.tensor.matmul(
                            ps2[c][DT:D1, :], w2t, h1s[:, csl(c)],
                            start=True, stop=True))
                        pe_mm2[(s, c)] = pe
                    if not last_step:
                        # next-step mm1 x0 part; ps1[c]/ps1b[c] WAR cleared
                        # by the act_h1/dve_h1b waits just above
                        mm1_group(s + 1, c, is_first=False)
                if not last_step:
                    for c in range(NCHUNK):
                        for t in range(s):
                            mm1_h2part(s + 1, c, t, last=False)
                        # term t == s arrives in two pieces when the h2
                        # activation is split across ACT and DVE; wait for
                        # both producers
                        twait(act_sem, act_h2[(s, c)])
                        if DD:
                            twait(dve_sem, dve_h2b[(s, c)])
                        pes = mm1_h2part(s + 1, c, s, last=True)
                        pe_mm1[(s + 1, c)] = pes[0]
                        pe_mm1b[(s + 1, c)] = pes[-1]
            for c in range(NCHUNK):
                if kk > 0:
                    twait(act_sem, act_h2[(kk - 1, c)])
                    if DD:
                        twait(dve_sem, dve_h2b[(kk - 1, c)])
                for g in range(c * (G // NCHUNK), (c + 1) * (G // NCHUNK)):
                    mm3_column(g)
            pe_mm3_last = pe
            twait(act_sem, act_sig)
            inc(nc.tensor.transpose(
                pst[0:G, 0:K], sig[0:K, 0:G], ident[0:K, 0:K]))
            pe_transpose = pe

        @block.scalar
        def _(scalar):
            scalar.dma_start(xcT[:, csl(1)], xd[:, csl(1)]).then_inc(
                sem_d[1], 16)
            zcell = nc.const_aps.aps[(mybir.dt.float32, 0.0)][0:1, 0:1]
            nc.scalar.activation(warm[0:1, 0:1], zcell, SIGMOID).then_inc(
                act_sem, 1)
            seen = 0

            def swait(val):
                nonlocal seen
                if val > seen:
                    seen = val
                    scalar.wait_ge(pe_sem, val)

            for s in range(kk):
                for c in range(NCHUNK):
                    lo = c * CHUNK
                    swait(pe_mm1[(s, c)])
                    nc.scalar.activation(
                        h1s[:, lo:lo + AH], ps1[c][:, 0:AH], PRELU,
                        bias=b1, alpha=0.01,
                    ).then_inc(act_sem, 1)
                for c in range(NCHUNK):
                    lo = c * CHUNK
                    swait(pe_mm2[(s, c)])
                    nc.scalar.activation(
                        h2s[s][DT:D1, lo:lo + AA], ps2[c][DT:D1, 0:AA],
                        PRELU, bias=b2, alpha=0.01,
                    ).then_inc(act_sem, 1)
            swait(pe_mm3_last)
            nc.scalar.activation(
                sig[0:K, 0:G], ps3[0:K, 0:G], SIGMOID, bias=obc[0:K, :],
            ).then_inc(act_sem, 1)

        @block.vector
        def _(vector):
            if fast_out:
                # the scatter-add src AP spans all 128 partitions; only
                # 0:G hold real products, the rest must be initialized
                # (they map to ignored negative indices).  Engines need
                # quadrant-aligned base partitions, so clear the full
                # column up front and let the scan overwrite rows 0:G.
                nc.vector.memset(prod[:, K - 1:K], 0.0).then_inc(
                    dve_sem, 1)
            # lrelu offload pairs: out = max(x + b, 0.01*(x + b))
            for s in range(kk):
                for c in range(NCHUNK):
                    if not HH:
                        continue
                    lo = c * CHUNK
                    tsl = slice((s * NCHUNK + c) * HH,
                                (s * NCHUNK + c + 1) * HH)
                    vector.wait_ge(pe_sem, pe_mm1b[(s, c)])
                    nc.vector.tensor_scalar(
                        out=tmph[:, tsl], in0=ps1b[c][:, 0:HH],
                        scalar1=b1, scalar2=0.01,
                        op0=mybir.AluOpType.add, op1=mybir.AluOpType.mult,
                    ).then_inc(dve_sem, 1)
                    vector.wait_ge(dve_sem, dve_h1b[(s, c)] - 1)
                    nc.vector.scalar_tensor_tensor(
                        out=h1s[:, lo + AH:lo + CHUNK],
                        in0=ps1b[c][:, 0:HH], scalar=b1,
                        in1=tmph[:, tsl],
                        op0=mybir.AluOpType.add, op1=mybir.AluOpType.max,
                    ).then_inc(dve_sem, 1)
                for c in range(NCHUNK):
                    if not DD:
                        continue
                    lo = c * CHUNK
                    tsl = slice((s * NCHUNK + c) * DD,
                                (s * NCHUNK + c + 1) * DD)
                    vector.wait_ge(pe_sem, pe_mm2b[(s, c)])
                    nc.vector.tensor_scalar(
                        out=tmp[DT:D1, tsl],
                        in0=psb[c][DT:D1, CHUNK - DD:CHUNK],
                        scalar1=b2, scalar2=0.01,
                        op0=mybir.AluOpType.add, op1=mybir.AluOpType.mult,
                    ).then_inc(dve_sem, 1)
                    # explicit completion wait: DVE RAW on tmp
                    vector.wait_ge(dve_sem, dve_h2b[(s, c)] - 1)
                    nc.vector.scalar_tensor_tensor(
                        out=h2s[s][DT:D1, lo + AA:lo + CHUNK],
                        in0=psb[c][DT:D1, CHUNK - DD:CHUNK], scalar=b2,
                        in1=tmp[DT:D1, tsl],
                        op0=mybir.AluOpType.add, op1=mybir.AluOpType.max,
                    ).then_inc(dve_sem, 1)
            # explicit ordering: the scan below rewrites rows 0:G of the
            # memset column
            vector.wait_ge(dve_sem, dve_scan - 1)
            vector.wait_ge(pe_sem, pe_transpose)
            nc.vector.tensor_tensor_scan(
                prod[0:G, 0:K], pst[0:G, 0:K], zrow,
                initial=1.0,
                op0=mybir.AluOpType.mult, op1=mybir.AluOpType.bypass,
            ).then_inc(dve_sem, 1)

        @block.gpsimd
        def _(gpsimd):
            gpsimd.dma_start(cs[:, :], const_d[:, :]).then_inc(sem_const, 16)
            if fast_out:
                from concourse import library_config
                nc.gpsimd.load_library(library_config.mlp)
                # zero the scatter-add target (completion-sem ordered
                # before the trigger below); reads the cpack zeros region
                gpsimd.wait_ge(sem_const, 16)
                gpsimd.dma_start(out_d[:, :], zrow).then_inc(sem_zero, 16)
                gpsimd.wait_ge(sem_idx, 16)
                nc.gpsimd.dma_scatter_add(
                    out_ap=out_d[:, 0:1],
                    in_ap=prod[:, K - 1:K],
                    idxs_ap=idxs_sb[:, :],
                    num_idxs=128,
                    num_idxs_reg=G,
                    elem_size=1,
                    elem_step=64,
                    prepare_only=True,
                    sem=sem_out,
                ).then_inc(sem_prep, 1)
                gpsimd.wait_ge(sem_prep, 1)
                gpsimd.wait_ge(sem_zero, 16)
                gpsimd.wait_ge(dve_sem, dve_scan)
                gpsimd.trigger_dma(1)

    return nc


def _pack_consts(w1, b1, w2, b2, ow, ob):
    cp = np.zeros((128, CW), np.float32)
    cp[:, C_ID:C_ID + 128] = np.eye(128, dtype=np.float32)
    bv = cp.view(np.uint8)
    bf = ml_dtypes.bfloat16
    bv[:, C_W1B * 4:C_W1B * 4 + 256] = np.ascontiguousarray(
        w1.T.astype(bf)).view(np.uint8)
    bv[:, C_W2B * 4:C_W2B * 4 + 128] = np.ascontiguousarray(
        w2.T.astype(bf)).view(np.uint8)
    bv[DT:D1, C_OWB * 4:C_OWB * 4 + 2] = np.ascontiguousarray(
        ow.reshape(D2, 1).astype(bf)).view(np.uint8)
    cp[:, C_B1] = b1
    cp[DT:D1, C_B2] = b2
    cp[:, C_OB] = np.float32(ob.reshape(())[()])
    return cp


def _scatter_idxs():
    # scatter-add index table: position j (column-major over [16, ncols])
    # maps input partition j; the first G positions target output row g=j,
    # the rest are the ignored -1
    idx = np.full((128, 8), -1, np.int16)
    idx[0:G, 0] = np.arange(G, dtype=np.int16)
    return idx


def _make_in_maps(towers, x, w1, b1, w2, b2, ow, ob):
    towers = np.asarray(towers, np.float32)
    x = np.asarray(x, np.float32)
    cpack = _pack_consts(
        np.asarray(w1, np.float32), np.asarray(b1, np.float32),
        np.asarray(w2, np.float32), np.asarray(b2, np.float32),
        np.asarray(ow, np.float32), np.asarray(ob, np.float32),
    )
    xcT = np.concatenate(
        [towers.reshape(N * K, DT), x.reshape(N * K, D2)], axis=1
    ).T.astype(ml_dtypes.bfloat16)          # [128, N*K]
    in_maps = []
    for i in range(N_CORES):
        sl = slice(i * R, (i + 1) * R)
        m = {"cpack": cpack, "xc0T": np.ascontiguousarray(xcT[:, sl])}
        if FAST_OUT:
            m["idxs"] = _scatter_idxs()
        in_maps.append(m)
    return in_maps


def kernel(towers, x, w1, b1, w2, b2, aw1, ab1, aw2, ab2, ow, ob, k):
    global LAST_RESULT
    kk = int(k)

    if kk not in _PROGRAM_CACHE:
        nc = _build_program(kk)
        # encode pseudo ISA instructions (library reload) to raw ISA bytes;
        # walrus codegen rejects un-encoded pseudo instructions ("ISA wrong
        # length").  No effect on CoreSim timing or numerics.
        mybir.codegen_inst_isa_subclasses(nc)
        _PROGRAM_CACHE[kk] = nc
    nc = _PROGRAM_CACHE[kk]

    in_maps = _make_in_maps(towers, x, w1, b1, w2, b2, ow, ob)
    res = run_bass_kernel_spmd(nc, in_maps, list(range(N_CORES)))
    LAST_RESULT = res
    # out[g, 0] = product of graph g of that core
    out = np.concatenate([
        np.asarray(res.results[i]["out"])[:, 0].reshape(G)
        for i in range(N_CORES)
    ])
    return out.astype(np.float32)
